# revision 40
# baseline (speedup 1.0000x reference)
"""Trainium2 Bass kernel for SSD MultiBox loss (nn_ModelLoss_5970004541458).

Strategy: data-parallel over batch (32 images -> 8 cores x 4 images).
Per core, everything over the prior dim (P=8732, padded to 8960 = 70*128)
runs on-device:
  - jaccard matching (16 boxes x 8960 priors) in bf16 in the LOG domain:
    overlap values only feed max/argmax/threshold comparisons, and ln is
    strictly monotone, so lov = Ln(inter+eps) - Ln(den) (both Lns on ACT)
    replaces the expensive DVE reciprocal.  Prior rows are pre-expanded
    across K on the host so every DVE tensor op has packed 2-byte operands
    (2x/4x mode).  Forced-assignment sentinels: fmask*(104+0.5k) - 100 =
    4+0.5k where forced (bf16-exact steps), -100 (below any lov) where not.
  - per-prior one-hot box gather via PE transpose + block-diag matmul (bf16)
  - CE: exp on ACT (bf16); class-sums via Pool half-add + short DVE reduce
  - hard-negative mining via a 2-level 16-way counting grid with bounded-
    error boundary correction (no sort), cen carried in bf16
All inputs are marshaled host-side into p-major layouts so every DMA is
contiguous per partition (128 large descriptors, not 8960 tiny ones).
Each core returns 16 partial sums; the host combines them into the loss.

This walrus build rejects: gpsimd partition_all_reduce/partition_broadcast,
custom-DVE ops (reciprocal_approx_*), gpsimd TT with broadcast APs or
comparison opcodes, EVENT_SEMAPHORE_RANGE_CLEAR.  Pool (gpsimd) is used
only for dense/strided add/mult/sub/copy.
"""
import sys

for _p in ("/opt/trn_rl_repo",):
    if _p not in sys.path:
        sys.path.insert(0, _p)

import numpy as np

import concourse.bass as bass
import concourse.tile as tile
from concourse import mybir
from concourse.bass_utils import run_bass_kernel_spmd

F32 = mybir.dt.float32
BF16 = mybir.dt.bfloat16
AX = mybir.AxisListType
OP = mybir.AluOpType
ACTF = mybir.ActivationFunctionType

B, P, C, K = 32, 8732, 81, 16
NCORES = 8
I = B // NCORES          # images per core = 4
PP = 8960                # padded priors = 70 * 128
T = PP // 128            # 70 prior tiles
T8 = 80                  # padded tile count for the m16 max tree
T2 = 72                  # padded tile count for 128-col transpose blocks
NB = T2 * K // 128       # 9 transpose blocks of 128 (t,k)-columns
NCH = 7                  # score chunks per image (10 tiles each)
CT = T // NCH            # tiles per chunk = 10
LN_THR = -1.0986122886681098   # ln(1/3); ov>=.5 <=> i/A>=1/3
KV0 = 104.0              # sentinel base: fmask*(104+0.5k) - 100 -> 4+0.5k
NQ = 5                   # gathered quantities per box (cx, cy, 5lnw, 5lnh, pad)

# f32 const blob column offsets
C_IO15 = 0
C_IV4 = C_IO15 + 16
C_PC4 = C_IV4 + T2 * 4
C_ONE = C_PC4 + T2 * 4
C_IDF = C_ONE + 1
C_O128 = C_IDF + 128
C_TOT = C_O128 + 128
# bf16 const blob column offsets (prior rows pre-expanded across K)
B_PX1 = 0
B_PY1 = B_PX1 + T * K
B_PX2 = B_PY1 + T * K
B_PY2 = B_PX2 + T * K
B_PAREA = B_PY2 + T * K
B_KV = B_PAREA + T * K
B_EPS = B_KV + T * K
B_TOT = B_EPS + 1

_bf16 = np.dtype("uint16")  # bf16 carried as uint16 bit pattern if ml_dtypes absent
try:
    import ml_dtypes

    _bf16 = np.dtype(ml_dtypes.bfloat16)
except ImportError:
    ml_dtypes = None


def _to_bf16(x: np.ndarray) -> np.ndarray:
    if ml_dtypes is not None:
        return x.astype(ml_dtypes.bfloat16)
    u = x.astype(np.float32).view(np.uint32)
    rounded = ((u >> 16) + ((u >> 15) & 1)).astype(np.uint32)
    return (rounded & 0xFFFF).astype(np.uint16)


def _fixup_module(nc: bass.Bass) -> None:
    """Adapt the Tile-generated module to this container's walrus build.

    - EVENT_SEMAPHORE_RANGE_CLEAR is rejected ("ISA wrong length"); the
      preceding Drain(is_reset_sema) already resets the same range, so drop it.
    - Seq-only instructions accept fewer sync waits than Tile emits; hoist
      excess waits onto NoOps placed immediately before (same engine, so
      program order preserves semantics).
    """
    import bass_rust

    for f in nc.m.functions:
        for blk in f.blocks:
            newl = []
            for ins in blk.instructions:
                if getattr(ins, "op_name", None) == "EVENT_SEMAPHORE_RANGE_CLEAR":
                    continue
                si = ins.sync_info
                maxw = 1
                if si is not None and si.on_wait and len(si.on_wait) > maxw:
                    waits = list(si.on_wait)
                    extra, keep = waits[:-maxw], waits[-maxw:]
                    for j in range(0, len(extra), 1):
                        nop = mybir.InstNoOp(
                            name=f"{ins.name}-wsplit{j}", ins=[], outs=[],
                            engine=ins.engine)
                        nop.sync_info = bass_rust.SyncInfo(
                            on_wait=[extra[j]], on_update=[])
                        newl.append(nop)
                    ins.sync_info = bass_rust.SyncInfo(
                        on_wait=keep,
                        on_update=list(si.on_update) if si.on_update else [])
                newl.append(ins)
            blk.instructions = newl


def build_nc(fixup: bool = True) -> bass.Bass:
    nc = bass.Bass()

    # p-major marshaled inputs: [*, 128, free] with contiguous per-partition rows
    d_scores = nc.dram_tensor("scores", [I, 128, T * C], BF16, kind="ExternalInput")
    d_locs = nc.dram_tensor("locs", [I, 128, T * 4], F32, kind="ExternalInput")
    d_cbf = nc.dram_tensor("cbf", [128, B_TOT], BF16, kind="ExternalInput")
    d_cf32 = nc.dram_tensor("cf32", [128, C_TOT], F32, kind="ExternalInput")
    # per-image free-layout box rows (bf16): 0 bx1 1 by1 2 bx2 3 by2 4 barea
    d_boxf = nc.dram_tensor("boxf", [I, 5, K], BF16, kind="ExternalInput")
    # per-image block-diagonal gather stationary [128, 8*NQ] (bf16)
    d_qblk = nc.dram_tensor("qblk", [I, 128, 8 * NQ], BF16, kind="ExternalInput")
    d_lblT = nc.dram_tensor("lblT", [I, K, 1], F32, kind="ExternalInput")
    # ln(parea + barea) per image, p-major bf16 (host-precomputed)
    d_lnA = nc.dram_tensor("lnA", [I, 128, T * K], BF16, kind="ExternalInput")
    d_ident = nc.dram_tensor("ident", [128, 128], BF16, kind="ExternalInput")
    d_ones_r = nc.dram_tensor("ones_r", [1, 128], F32, kind="ExternalInput")
    d_onesb = nc.dram_tensor("onesb", [1, 128], BF16, kind="ExternalInput")
    d_iota81 = nc.dram_tensor("iota81", [K, C], F32, kind="ExternalInput")
    # out row layout (single partition): [np0..3, box0..3, cep0..3, mine0..3]
    d_out = nc.dram_tensor("out", [1, 16], F32, kind="ExternalOutput")

    from contextlib import ExitStack

    with tile.TileContext(nc) as tc, ExitStack() as es:
        cpool = es.enter_context(tc.tile_pool(name="consts", bufs=1))
        spool = es.enter_context(tc.tile_pool(name="scores", bufs=4))
        prepool = es.enter_context(tc.tile_pool(name="prein", bufs=4))
        wpool = es.enter_context(tc.tile_pool(name="work", bufs=2))
        epool = es.enter_context(tc.tile_pool(name="exp", bufs=3))
        bpool = es.enter_context(tc.tile_pool(name="batched", bufs=1))
        pp_t = es.enter_context(tc.tile_pool(name="ps_t", bufs=1, space="PSUM"))
        pp_sel = es.enter_context(tc.tile_pool(name="ps_sel", bufs=1, space="PSUM"))
        pp_u = es.enter_context(tc.tile_pool(name="ps_u", bufs=1, space="PSUM"))
        pp_r = es.enter_context(tc.tile_pool(name="ps_r", bufs=2, space="PSUM"))
        rpool = es.enter_context(tc.tile_pool(name="redsb", bufs=4))

        # ------- constants + all per-image loads, DMA-issue spread over
        # ------- SP/ACT/Pool queues (engine-synchronous issue serializes)
        cbf = cpool.tile([128, B_TOT], BF16, tag="cbf")
        cf32 = cpool.tile([128, C_TOT], F32, tag="cf32")
        ident = cpool.tile([128, 128], BF16, tag="ident")
        ones_r = cpool.tile([1, 128], F32, tag="ones_r")
        onesb = cpool.tile([1, 128], BF16, tag="onesb")
        iota81 = cpool.tile([K, C], F32, tag="iota81")

        sres_l = [spool.tile([128, T, C], BF16, tag="sres", name=f"sres{i}")
                  for i in range(I)]
        l4_l = [prepool.tile([128, T2, 4], F32, tag="l4", name=f"l4{i}")
                for i in range(I)]
        boxf_l = [prepool.tile([1, 5, K], BF16, tag="boxf", name=f"boxf{i}")
                  for i in range(I)]
        qblk_l = [prepool.tile([128, 8 * NQ], BF16, tag="qblk",
                               name=f"qblk{i}") for i in range(I)]
        lbl_l = [prepool.tile([K, 1], F32, tag="lbl16", name=f"lbl{i}")
                 for i in range(I)]
        lnA_l = [prepool.tile([128, T, K], BF16, tag="lnA", name=f"lnA{i}")
                 for i in range(I)]

        # ACT queue: image-0 box row + onesb first (bb broadcast unblocks)
        nc.scalar.dma_start(out=boxf_l[0][:], in_=d_boxf[0, :, :][None])
        nc.scalar.dma_start(out=onesb[:], in_=d_onesb[:, :])
        for i in range(1, I):
            nc.scalar.dma_start(out=boxf_l[i][:], in_=d_boxf[i, :, :][None])
        nc.scalar.dma_start(out=sres_l[0][:].rearrange("p t c -> p (t c)"),
                            in_=d_scores[0, :, :])
        nc.scalar.dma_start(out=ident[:], in_=d_ident[:, :])
        nc.scalar.dma_start(out=iota81[:], in_=d_iota81[:, :])
        nc.scalar.dma_start(out=ones_r[:], in_=d_ones_r[:, :])
        # SP queue: jaccard consts, then remaining scores staged with the
        # small per-image tensors they unblock
        nc.sync.dma_start(out=cbf[:], in_=d_cbf[:, :])
        nc.sync.dma_start(out=lnA_l[0][:].rearrange("p t k -> p (t k)"),
                          in_=d_lnA[0, :, :])
        nc.sync.dma_start(out=sres_l[1][:].rearrange("p t c -> p (t c)"),
                          in_=d_scores[1, :, :])
        nc.sync.dma_start(out=cf32[:], in_=d_cf32[:, :])
        for i in range(1, I):
            nc.sync.dma_start(out=lnA_l[i][:].rearrange("p t k -> p (t k)"),
                              in_=d_lnA[i, :, :])
        for i in range(I):
            nc.vector.memset(l4_l[i][:, T:, :], 0.0)
        for i in range(I):
            nc.sync.dma_start(
                out=l4_l[i][:, :T, :].rearrange("p t d -> p (t d)"),
                in_=d_locs[i, :, :])
            nc.sync.dma_start(out=qblk_l[i][:], in_=d_qblk[i, :, :])
            nc.sync.dma_start(out=lbl_l[i][:], in_=d_lblT[i, :, :])
            if i >= 2:
                nc.sync.dma_start(
                    out=sres_l[i][:].rearrange("p t c -> p (t c)"),
                    in_=d_scores[i, :, :])

        def prow(off):  # bf16 pre-expanded prior row view [128, T, K]
            return cbf[:, off:off + T * K].rearrange("p (t k) -> p t k", k=K)

        pxe = {nm: prow(off) for nm, off in
               [("px1", B_PX1), ("py1", B_PY1), ("px2", B_PX2),
                ("py2", B_PY2), ("parea", B_PAREA)]}
        kvb = prow(B_KV)
        eps_b = cbf[:, B_EPS:B_EPS + 1]
        io15 = cf32[:, C_IO15:C_IV4]
        iv4 = cf32[:, C_IV4:C_PC4].rearrange("p (t d) -> p t d", d=4)
        pc4 = cf32[:, C_PC4:C_ONE].rearrange("p (t d) -> p t d", d=4)
        ones_p = cf32[:, C_ONE:C_IDF]
        identf = cf32[:, C_IDF:C_O128]
        ones128 = cf32[:, C_O128:C_TOT]

        def rowsum(dst_row_ap, src_ap, n):
            """[P, n] f32 -> [1, n] partition sum written to dst_row_ap."""
            ps = pp_r.tile([128, 128], F32, tag="red_bc")
            nc.tensor.matmul(ps[0:1, :n], lhsT=ones_p[:src_ap.shape[0], :],
                             rhs=src_ap, start=True, stop=True)
            nc.vector.tensor_copy(dst_row_ap, ps[0:1, :n])

        def bcast_row(dst_ap, row_ap, n):
            """[1, n] f32 -> [128, n] replicated (dst may be bf16)."""
            ps = pp_r.tile([128, 128], F32, tag="red_bc")
            nc.tensor.matmul(ps[:, :n], lhsT=ones_r[:], rhs=row_ap,
                             start=True, stop=True)
            nc.vector.tensor_copy(dst_ap, ps[:, :n])

        def allreduce_sum(dst_ap, src_ap, n):
            ps = pp_r.tile([128, 128], F32, tag="red_bc")
            nc.tensor.matmul(ps[:, :n], lhsT=ones128, rhs=src_ap,
                             start=True, stop=True)
            nc.vector.tensor_copy(dst_ap, ps[:, :n])

        def maxreduce_row(dst_row_ap, src_ap, n):
            """[128, n] f32 -> [1, n] partition max written to dst_row_ap."""
            ps = pp_r.tile([128, 128], F32, tag="red_bc")
            nc.tensor.transpose(ps[:n, :], src_ap, identf)
            tsb = rpool.tile([128, 128], F32, tag="red_tsb")
            nc.vector.tensor_copy(tsb[:n, :], ps[:n, :])
            mx = rpool.tile([128, 1], F32, tag="red_mx")
            nc.vector.tensor_reduce(out=mx[:n, :], in_=tsb[:n, :],
                                    axis=AX.X, op=OP.max)
            ps2 = pp_r.tile([128, 128], F32, tag="red_bc")
            nc.tensor.transpose(ps2[0:1, :n], mx[:n, :], identf[:n, :n])
            nc.vector.tensor_copy(dst_row_ap, ps2[0:1, :n])

        # batched buffers [128, I, T]
        se4 = bpool.tile([128, I, T], BF16, tag="se4")
        lse4 = bpool.tile([128, I, T], F32, tag="lse4")
        cen4 = bpool.tile([128, I, T], BF16, tag="cen4")
        np4 = bpool.tile([128, I], F32, tag="np4")
        k34 = bpool.tile([128, I], F32, tag="k34")
        cnt = bpool.tile([128, I, 16], F32, tag="cnt")
        cntr = bpool.tile([128, I, 16], F32, tag="cntr")
        lo4 = bpool.tile([128, I], F32, tag="lo4")
        lop = bpool.tile([128, I], F32, tag="lop")
        hi4 = bpool.tile([128, I], F32, tag="hi4")
        scadd = bpool.tile([128, I, 4], F32, tag="scadd")   # fs, cn, lps, box
        scrow = bpool.tile([1, I, 4], F32, tag="scrow")
        bm4 = bpool.tile([128, I], F32, tag="bm4")
        bmrow = bpool.tile([1, I], F32, tag="bmrow")
        uf4 = bpool.tile([1, I], F32, tag="uf4")
        out_sb = bpool.tile([1, 16], F32, tag="out_sb")

        def emit_mining(i):
            # ---------------- mining (per image, pipelined) --------------
            nc.vector.tensor_scalar(out=k34[:, i:i + 1], in0=np4[:, i:i + 1],
                                    scalar1=3.0, scalar2=None, op0=OP.mult)
            allreduce_sum(cntr[:, i, :], cnt[:, i, :], 16)
            # lo = (#edges with count >= k) - 1   (edges j = 0..15)
            ge16 = bpool.tile([128, 16], F32, tag="ge16")
            nc.vector.tensor_scalar(out=ge16[:], in0=cntr[:, i, :],
                                    scalar1=k34[:, i:i + 1], scalar2=None,
                                    op0=OP.is_ge)
            nc.vector.tensor_reduce(out=lo4[:, i:i + 1], in_=ge16[:],
                                    axis=AX.X, op=OP.add)
            nc.vector.tensor_scalar(out=lo4[:, i:i + 1], in0=lo4[:, i:i + 1],
                                    scalar1=-1.0, scalar2=None, op0=OP.add)
            nc.vector.tensor_scalar(out=lop[:, i:i + 1], in0=lo4[:, i:i + 1],
                                    scalar1=1.0 / 16, scalar2=None, op0=OP.add)
            # level 2: thresholds lo + m/16 (io15 has (1..15)/16 then +999)
            thr2 = bpool.tile([128, 16], F32, tag="thr2")
            nc.vector.tensor_scalar(out=thr2[:], in0=io15,
                                    scalar1=lo4[:, i:i + 1], scalar2=None,
                                    op0=OP.add)
            c2 = bpool.tile([128, 16], F32, tag="c2")
            msci2 = wpool.tile([128, T], BF16, tag="msci2")
            for m in range(16):
                nc.vector.tensor_scalar(out=msci2[:], in0=cen4[:, i, :],
                                        scalar1=thr2[:, m:m + 1], scalar2=None,
                                        op0=OP.is_gt, op1=OP.add,
                                        accum_out=c2[:, m:m + 1])
            c2r = bpool.tile([128, 16], F32, tag="c2r")
            allreduce_sum(c2r[:], c2[:], 16)
            mc = bpool.tile([128, 1], F32, tag="mc")
            nc.vector.tensor_scalar(out=ge16[:], in0=c2r[:],
                                    scalar1=k34[:, i:i + 1], scalar2=None,
                                    op0=OP.is_ge, op1=OP.add, accum_out=mc[:])
            nc.vector.tensor_scalar(out=hi4[:, i:i + 1], in0=mc[:],
                                    scalar1=1.0 / 16, scalar2=lop[:, i:i + 1],
                                    op0=OP.mult, op1=OP.add)
            # F(hi), count(hi), boundary max
            fsc = bpool.tile([128, T], BF16, tag="fsc")
            nc.vector.scalar_tensor_tensor(
                out=fsc[:], in0=cen4[:, i, :], scalar=hi4[:, i:i + 1],
                in1=cen4[:, i, :], op0=OP.is_gt, op1=OP.mult,
                accum_out=scadd[:, i, 0:1])
            nc.vector.tensor_scalar(out=fsc[:], in0=cen4[:, i, :],
                                    scalar1=hi4[:, i:i + 1], scalar2=None,
                                    op0=OP.is_gt, op1=OP.add,
                                    accum_out=scadd[:, i, 1:2])
            nc.vector.scalar_tensor_tensor(
                out=fsc[:], in0=cen4[:, i, :], scalar=hi4[:, i:i + 1],
                in1=cen4[:, i, :], op0=OP.is_le, op1=OP.mult)
            nc.vector.tensor_reduce(out=bm4[:, i:i + 1], in_=fsc[:],
                                    axis=AX.X, op=OP.max)

        bb_l = []
        for i in range(I):
            bbt = prepool.tile([128, 5, K], BF16, tag="bb", name=f"bb{i}")
            ps_bb = pp_r.tile([128, 128], F32, tag="red_bc",
                              name=f"psbb{i}")
            nc.tensor.matmul(ps_bb[:, :5 * K], lhsT=onesb[:],
                             rhs=boxf_l[i][:].rearrange("p a k -> p (a k)"),
                             start=True, stop=True)
            nc.vector.tensor_copy(bbt[:].rearrange("p a k -> p (a k)"),
                                  ps_bb[:, :5 * K])
            bb_l.append(bbt)

        for i in range(I):
            sres = sres_l[i]
            l4 = l4_l[i]
            qblk = qblk_l[i]
            lbl16 = lbl_l[i]
            bb = bb_l[i]

            def bcast_b(row):  # [128, K] box row -> [128, T, K] AP (packed k)
                return bb[:, row, :][:, None, :].broadcast_to([128, T, K])

            # ---------------- jaccard (log domain, bf16) -----------------
            # ordering uses i/(pa+ba): x/(A-x) is a monotone bijection of
            # x/A, so argmax/threshold semantics match IoU with thr ln(1/3)
            lov80 = wpool.tile([128, T8, K], BF16, tag="lov80")
            nc.vector.memset(lov80[:, T:, :], -100.0)
            lov = lov80[:, :T, :]
            ltx = wpool.tile([128, T, K], BF16, tag="ltx")
            lty = wpool.tile([128, T, K], BF16, tag="lty")
            w0 = wpool.tile([128, T, K], BF16, tag="w0")
            h0 = wpool.tile([128, T, K], BF16, tag="h0")
            wr = wpool.tile([128, T, K], BF16, tag="wr")
            hr = wpool.tile([128, T, K], BF16, tag="hr")
            inter = wpool.tile([128, T, K], BF16, tag="inter")

            nc.vector.tensor_tensor(out=ltx[:], in0=pxe["px1"],
                                    in1=bcast_b(0), op=OP.max)
            nc.vector.tensor_tensor(out=lty[:], in0=pxe["py1"],
                                    in1=bcast_b(1), op=OP.max)
            nc.vector.tensor_tensor(out=w0[:], in0=pxe["px2"],
                                    in1=bcast_b(2), op=OP.min)
            nc.vector.tensor_tensor(out=h0[:], in0=pxe["py2"],
                                    in1=bcast_b(3), op=OP.min)
            nc.vector.tensor_sub(wr[:], w0[:], ltx[:])
            nc.vector.tensor_sub(hr[:], h0[:], lty[:])
            nc.scalar.activation(wr[:], wr[:], ACTF.Relu)
            nc.scalar.activation(hr[:], hr[:], ACTF.Relu)
            nc.vector.tensor_mul(inter[:], wr[:], hr[:])
            nc.scalar.activation(inter[:], inter[:], ACTF.Ln, bias=eps_b)
            nc.vector.tensor_sub(lov, inter[:], lnA_l[i][:])

            # ---------------- matching pass 2 ----------------
            # per-box max over priors: dense max tree (80 = 2*2*2*2*5)
            tm1 = wpool.tile([128, 40, K], BF16, tag="tm1")
            nc.vector.tensor_tensor(out=tm1[:], in0=lov80[:, :40, :],
                                    in1=lov80[:, 40:, :], op=OP.max)
            tm2 = wpool.tile([128, 20, K], BF16, tag="tm2")
            nc.vector.tensor_tensor(out=tm2[:], in0=tm1[:, :20, :],
                                    in1=tm1[:, 20:, :], op=OP.max)
            tm3 = wpool.tile([128, 10, K], BF16, tag="tm3")
            nc.vector.tensor_tensor(out=tm3[:], in0=tm2[:, :10, :],
                                    in1=tm2[:, 10:, :], op=OP.max)
            tm4 = wpool.tile([128, 5, K], BF16, tag="tm4")
            nc.vector.tensor_tensor(out=tm4[:], in0=tm3[:, :5, :],
                                    in1=tm3[:, 5:, :], op=OP.max)
            m16 = wpool.tile([128, K], F32, tag="m16")
            nc.vector.tensor_reduce(
                out=m16[:], in_=tm4[:].rearrange("p t k -> p k t"),
                axis=AX.X, op=OP.max)
            m16row = wpool.tile([1, K], F32, tag="m16row")
            maxreduce_row(m16row[:], m16[:], K)
            m16rb = wpool.tile([128, K], BF16, tag="m16rb")
            bcast_row(m16rb[:], m16row[:], K)
            fmask = wpool.tile([128, T, K], BF16, tag="fmask")
            nc.vector.tensor_tensor(
                out=fmask[:], in0=lov,
                in1=m16rb[:][:, None, :].broadcast_to([128, T, K]),
                op=OP.is_equal)
            ovf = wpool.tile([128, T, K], BF16, tag="ovf")
            fm2 = wpool.tile([128, T, K], BF16, tag="fm2")
            nc.vector.tensor_mul(fm2[:], fmask[:], kvb)
            nc.vector.scalar_tensor_tensor(
                out=ovf[:], in0=fm2[:], scalar=-100.0, in1=lov,
                op0=OP.add, op1=OP.max)
            # per-prior max over k: dense tree on the packed innermost dim
            ms1 = wpool.tile([128, T, 8], BF16, tag="ms1")
            nc.vector.tensor_tensor(out=ms1[:], in0=ovf[:, :, 0:8],
                                    in1=ovf[:, :, 8:16], op=OP.max)
            ms2 = wpool.tile([128, T, 4], BF16, tag="ms2")
            nc.vector.tensor_tensor(out=ms2[:], in0=ms1[:, :, 0:4],
                                    in1=ms1[:, :, 4:8], op=OP.max)
            ms3 = wpool.tile([128, T, 2], BF16, tag="ms3")
            nc.vector.tensor_tensor(out=ms3[:], in0=ms2[:, :, 0:2],
                                    in1=ms2[:, :, 2:4], op=OP.max)
            pm = wpool.tile([128, T], BF16, tag="pm")
            nc.vector.tensor_tensor(out=pm[:], in0=ms3[:, :, 0],
                                    in1=ms3[:, :, 1], op=OP.max)
            # pmz = pm where positive else pm+1 (matches nothing): fuses the
            # one-hot and the pos mask into a single is_eq
            pmz = wpool.tile([128, T], BF16, tag="pmz")
            nc.vector.scalar_tensor_tensor(
                out=pmz[:], in0=pm[:], scalar=LN_THR, in1=pm[:],
                op0=OP.is_lt, op1=OP.add)
            wm72 = wpool.tile([128, T2 * K], BF16, tag="wm72")
            nc.vector.memset(wm72[:, T * K:], 0.0)
            wmat = wm72[:, :T * K].rearrange("p (t k) -> p t k", k=K)
            nc.vector.tensor_tensor(
                out=wmat, in0=ovf[:],
                in1=pmz[:][:, :, None].broadcast_to([128, T, K]),
                op=OP.is_equal)
            pos72 = wpool.tile([128, T2], F32, tag="pos72")
            nc.vector.memset(pos72[:, T:], 0.0)
            npt = wpool.tile([128, 1], F32, tag="npt")
            nc.vector.tensor_scalar(out=pos72[:, :T], in0=pm[:],
                                    scalar1=LN_THR, scalar2=None,
                                    op0=OP.is_ge, op1=OP.add, accum_out=npt[:])
            allreduce_sum(np4[:, i:i + 1], npt[:], 1)

            if i > 0:
                emit_mining(i - 1)

            # ---------------- box gather via PE ----------------
            ohT_ps = pp_t.tile([128, NB, 128], BF16, tag="ohT")
            for b in range(NB):
                nc.tensor.transpose(
                    ohT_ps[:, b, :],
                    wm72[:, b * 128:(b + 1) * 128],
                    ident[:])
            ohT_sb = wpool.tile([128, NB * 128], BF16, tag="ohT_sb")
            nc.scalar.copy(ohT_sb[:], ohT_ps[:].rearrange("p b n -> p (b n)"))

            sel_ps = pp_sel.tile([8 * NQ, NB, 128], F32, tag="sel")
            for b in range(NB):
                nc.tensor.matmul(sel_ps[:, b, :], lhsT=qblk[:],
                                 rhs=ohT_sb[:, b * 128:(b + 1) * 128],
                                 start=True, stop=True)
            sel_sb = wpool.tile([8 * NQ, NB * 128], BF16, tag="sel_sb")
            nc.scalar.copy(sel_sb[:], sel_ps[:].rearrange("p b n -> p (b n)"))
            bk_ps = pp_t.tile([128, NB, 8 * NQ], BF16, tag="ohT")
            for b in range(NB):
                nc.tensor.transpose(
                    bk_ps[:, b, :],
                    sel_sb[:, b * 128:(b + 1) * 128],
                    ident[:8 * NQ, :8 * NQ])
            selq = wpool.tile([128, NB * 8 * NQ], BF16, tag="selq")
            nc.scalar.copy(selq[:], bk_ps[:].rearrange("p b n -> p (b n)"))
            # selq[p, (blk*40 + tb*5 + q)] = sel_q at t = blk*8+tb
            sel4 = selq[:].rearrange("p (t q) -> p t q", q=NQ)[:, :, 0:4]

            # ---------------- box L1 (Pool chain + ACT abs-accum) ---------
            lp4 = wpool.tile([128, T2, 4], F32, tag="lp4")
            nc.gpsimd.tensor_add(lp4[:], l4[:], pc4)
            tb1 = wpool.tile([128, T2, 4], F32, tag="tb1")
            nc.gpsimd.tensor_mul(tb1[:], sel4, iv4)
            nc.gpsimd.tensor_sub(tb1[:], lp4[:], tb1[:])
            nc.vector.tensor_tensor(
                out=tb1[:], in0=tb1[:],
                in1=pos72[:][:, :, None].broadcast_to([128, T2, 4]),
                op=OP.mult)
            nc.scalar.activation(tb1[:], tb1[:], ACTF.Abs,
                                 accum_out=scadd[:, i, 3:4])

            # ---------------- U matrix (score at label) ----------------
            u_ps = pp_u.tile([K, C], F32, tag="u")
            for t_ in range(T):
                nc.tensor.matmul(u_ps[:], lhsT=wmat[:, t_, :],
                                 rhs=sres[:, t_, :],
                                 start=(t_ == 0), stop=(t_ == T - 1))
            u_sb = wpool.tile([K, C], F32, tag="u_sb")
            nc.scalar.copy(u_sb[:], u_ps[:])
            ufx = wpool.tile([K, C], F32, tag="ufx")
            ufa = wpool.tile([K, 1], F32, tag="ufa")
            nc.vector.scalar_tensor_tensor(
                out=ufx[:], in0=iota81[:], scalar=lbl16[:], in1=u_sb[:],
                op0=OP.is_equal, op1=OP.mult, accum_out=ufa[:])
            rowsum(uf4[:, i:i + 1], ufa[:], 1)

            # ------- CE: exp (ACT) + class sums (Pool half-adds + DVE) ----
            for ch in range(NCH):
                et = epool.tile([128, CT, C], BF16, tag="exps")
                nc.scalar.activation(
                    et[:], sres[:, ch * CT:(ch + 1) * CT, :], ACTF.Exp)
                et2 = epool.tile([128, CT, 40], BF16, tag="et2")
                sl = se4[:, i, ch * CT:(ch + 1) * CT]
                with nc.allow_low_precision("bf16 class sums"):
                    nc.gpsimd.tensor_add(et2[:], et[:, :, 0:40],
                                         et[:, :, 40:80])
                    nc.vector.tensor_reduce(out=sl, in_=et2[:], axis=AX.X,
                                            op=OP.add)
                    nc.vector.tensor_add(sl, sl, et[:, :, 80])
            # lse per image, ce0, cen = (1-pos)*ce0 fused
            nc.scalar.activation(lse4[:, i, :], se4[:, i, :], ACTF.Ln)
            ce0 = wpool.tile([128, T], F32, tag="ce0")
            nc.vector.tensor_sub(ce0[:], lse4[:, i, :], sres[:, :, 0])
            nc.vector.scalar_tensor_tensor(
                out=cen4[:, i, :], in0=pos72[:, :T], scalar=0.5,
                in1=ce0[:], op0=OP.is_lt, op1=OP.mult)
            msci = wpool.tile([128, T], BF16, tag="msci")
            for j in range(16):
                nc.vector.tensor_scalar(out=msci[:], in0=cen4[:, i, :],
                                        scalar1=float(j), scalar2=None,
                                        op0=OP.is_gt, op1=OP.add,
                                        accum_out=cnt[:, i, j:j + 1])
            lpst = wpool.tile([128, T], F32, tag="lpst")
            nc.vector.scalar_tensor_tensor(
                out=lpst[:], in0=pos72[:, :T], scalar=1.0,
                in1=lse4[:, i, :], op0=OP.mult, op1=OP.mult,
                accum_out=scadd[:, i, 2:3])

        emit_mining(I - 1)

        rowsum(scrow[:].rearrange("p i s -> p (i s)"),
               scadd[:].rearrange("p i s -> p (i s)"), I * 4)
        maxreduce_row(bmrow[:], bm4[:], I)

        # ---------------- final combine (partition 0) ----------------
        r4 = bpool.tile([1, I], F32, tag="r4")
        nc.vector.tensor_sub(r4[:], k34[0:1, :], scrow[:, :, 1])
        nc.vector.tensor_mul(r4[:], r4[:], bmrow[:])
        nc.vector.tensor_add(r4[:], r4[:], scrow[:, :, 0])   # mine sums
        cep = bpool.tile([1, I], F32, tag="cep")
        nc.vector.tensor_sub(cep[:], scrow[:, :, 2], uf4[:])  # ce_pos sums
        nc.vector.tensor_copy(out_sb[:, 0:4], np4[0:1, :])
        nc.vector.tensor_copy(out_sb[:, 4:8], scrow[:, :, 3])
        nc.vector.tensor_copy(out_sb[:, 8:12], cep[:])
        nc.vector.tensor_copy(out_sb[:, 12:16], r4[:])
        nc.sync.dma_start(out=d_out[:, :], in_=out_sb[:])

    if fixup:
        _fixup_module(nc)
    return nc


def prepare_inputs(predicted_locs, predicted_scores, boxes, labels,
                   priors_centers):
    """Shard + marshal the full inputs into 8 per-core in_maps (p-major)."""
    predicted_locs = np.asarray(predicted_locs, np.float32)
    predicted_scores = np.asarray(predicted_scores, np.float32)
    boxes = np.asarray(boxes, np.float32)
    labels_f = np.asarray(labels).astype(np.float32)
    priors = np.asarray(priors_centers, np.float32)

    npad = PP - P
    # scores: pad rows have class0=0, others -50 -> lse=0, S0=0, ce0=0 exactly
    pad_scores = np.full((B, npad, C), -50.0, np.float32)
    pad_scores[:, :, 0] = 0.0
    scores_p = np.concatenate([predicted_scores, pad_scores], axis=1)
    # p-major: [B, 128, T*C]
    scores_pm = np.ascontiguousarray(
        scores_p.reshape(B, T, 128, C).transpose(0, 2, 1, 3)
    ).reshape(B, 128, T * C)
    scores_bf = _to_bf16(scores_pm)
    locs_p = np.concatenate(
        [predicted_locs, np.zeros((B, npad, 4), np.float32)], axis=1)
    locs_pm = np.ascontiguousarray(
        locs_p.reshape(B, T, 128, 4).transpose(0, 2, 1, 3)
    ).reshape(B, 128, T * 4)

    # prior rows pre-expanded across K (p-major, bf16)
    pad_pri = np.tile(np.array([-100.0, -100.0, 1.0, 1.0], np.float32),
                      (npad, 1))
    pri = np.concatenate([priors, pad_pri], axis=0)
    pcx, pcy, pw, ph = pri[:, 0], pri[:, 1], pri[:, 2], pri[:, 3]

    def pm_grid(v):  # [PP] -> [128, T]
        return np.ascontiguousarray(v.astype(np.float32).reshape(T, 128).T)

    def pexp(v):  # [PP] -> [128, T*K] expanded across K
        return np.repeat(pm_grid(v)[:, :, None], K, axis=2).reshape(128, T * K)

    kv = np.tile(KV0 + 0.5 * np.arange(K, dtype=np.float32), (128, T))
    eps_b = np.full((128, 1), 1e-20, np.float32)
    cbf = _to_bf16(np.concatenate(
        [pexp(pcx - pw / 2), pexp(pcy - ph / 2), pexp(pcx + pw / 2),
         pexp(pcy + ph / 2), pexp(pw * ph), kv, eps_b], axis=1))
    assert cbf.shape[1] == B_TOT

    # iv4/pc4 [128, T2, 4] host-assembled (d = x, y, w, h; tail zero)
    iv4 = np.zeros((128, T2, 4), np.float32)
    pc4 = np.zeros((128, T2, 4), np.float32)
    iv4[:, :T, 0] = pm_grid(10.0 / pw)
    iv4[:, :T, 1] = pm_grid(10.0 / ph)
    iv4[:, :T, 2] = 1.0
    iv4[:, :T, 3] = 1.0
    pc4[:, :T, 0] = pm_grid(pcx * (10.0 / pw))
    pc4[:, :T, 1] = pm_grid(pcy * (10.0 / ph))
    pc4[:, :T, 2] = pm_grid(5.0 * np.log(pw))
    pc4[:, :T, 3] = pm_grid(5.0 * np.log(ph))

    io15 = np.tile(np.concatenate([np.arange(1, 16, dtype=np.float32) / 16.0,
                                   [999.0]]), (128, 1))
    ones_p = np.ones((128, 1), np.float32)
    identf = np.eye(128, dtype=np.float32)
    cf32 = np.concatenate(
        [io15, iv4.reshape(128, T2 * 4), pc4.reshape(128, T2 * 4),
         ones_p, identf, np.ones((128, 128), np.float32)],
        axis=1).astype(np.float32)
    assert cf32.shape[1] == C_TOT

    bx1, by1, bx2, by2 = (boxes[:, :, d] for d in range(4))
    barea = (bx2 - bx1) * (by2 - by1)
    boxf = _to_bf16(np.stack([bx1, by1, bx2, by2, barea], axis=1))
    # lnA[b, p, t, k] = ln(parea[p,t] + barea[b,k]) (p-major)
    parea_pm = pm_grid(pw * ph)                               # [128, T]
    lnA = _to_bf16(np.log(
        parea_pm[None, :, :, None] + barea[:, None, None, :].astype(np.float64)
    ).reshape(B, 128, T * K))
    q5 = np.stack([
        (bx1 + bx2) / 2, (by1 + by2) / 2,
        5.0 * np.log(bx2 - bx1), 5.0 * np.log(by2 - by1),
        np.zeros_like(bx1),
    ], axis=2).astype(np.float32)                           # [B, K, 5]
    qblk = np.zeros((B, 128, 8 * NQ), np.float32)
    for tb in range(8):
        qblk[:, tb * K:(tb + 1) * K, tb * NQ:(tb + 1) * NQ] = q5
    qblk = _to_bf16(qblk)

    ident = _to_bf16(np.eye(128, dtype=np.float32))
    ones_r = np.ones((1, 128), np.float32)
    onesb = _to_bf16(np.ones((1, 128), np.float32))
    iota81 = np.tile(np.arange(C, dtype=np.float32), (K, 1))

    in_maps = []
    for c in range(NCORES):
        sl = slice(c * I, (c + 1) * I)
        in_maps.append({
            "scores": scores_bf[sl],
            "locs": locs_pm[sl],
            "cbf": cbf,
            "cf32": cf32,
            "boxf": boxf[sl],
            "qblk": qblk[sl],
            "lblT": labels_f[sl][:, :, None],
            "lnA": lnA[sl],
            "ident": ident,
            "ones_r": ones_r,
            "onesb": onesb,
            "iota81": iota81,
        })
    return in_maps


def combine_outputs(outs):
    """outs: list of 8 per-core [1,16] arrays -> scalar loss."""
    parts = np.concatenate([o.reshape(4, 4) for o in outs], axis=1)  # [4, 32]
    n_pos_total = parts[0].sum()
    box_sum = parts[1].sum()
    class_sum = parts[2].sum() + parts[3].sum()
    loss = class_sum / n_pos_total + box_sum / (n_pos_total * 4.0)
    return np.float32(loss)


_NC_CACHE = {}


def kernel(predicted_locs, predicted_scores, boxes, labels, priors_centers):
    if "nc" not in _NC_CACHE:
        _NC_CACHE["nc"] = build_nc()
    nc = _NC_CACHE["nc"]
    in_maps = prepare_inputs(predicted_locs, predicted_scores, boxes, labels,
                             priors_centers)
    res = run_bass_kernel_spmd(nc, in_maps, list(range(NCORES)))
    outs = [res.results[c]["out"] for c in range(NCORES)]
    return combine_outputs(outs)


if __name__ == "__main__":
    import reference as R

    inputs = {k: np.asarray(v) for k, v in R.setup_inputs().items()}
    print("loss =", kernel(**inputs))


# revision 41
# speedup vs baseline: 1.1048x; 1.1048x over previous
"""Trainium2 Bass kernel for SSD MultiBox loss (nn_ModelLoss_5970004541458).

Strategy: data-parallel over batch (32 images -> 8 cores x 4 images).
Per core, everything over the prior dim (P=8732, padded to 8960 = 70*128)
runs on-device:
  - jaccard matching (16 boxes x 8960 priors) in bf16 in the LOG domain:
    overlap values only feed max/argmax/threshold comparisons, and ln is
    strictly monotone, so lov = Ln(inter+eps) - Ln(den) (both Lns on ACT)
    replaces the expensive DVE reciprocal.  Prior rows are pre-expanded
    across K on the host so every DVE tensor op has packed 2-byte operands
    (2x/4x mode).  Forced-assignment sentinels: fmask*(104+0.5k) - 100 =
    4+0.5k where forced (bf16-exact steps), -100 (below any lov) where not.
  - per-prior one-hot box gather via PE transpose + block-diag matmul (bf16)
  - CE: exp on ACT (bf16); class-sums via Pool half-add + short DVE reduce
  - hard-negative mining via a 2-level 16-way counting grid with bounded-
    error boundary correction (no sort), cen carried in bf16
All inputs are marshaled host-side into p-major layouts so every DMA is
contiguous per partition (128 large descriptors, not 8960 tiny ones).
Each core returns 16 partial sums; the host combines them into the loss.

This walrus build rejects: gpsimd partition_all_reduce/partition_broadcast,
custom-DVE ops (reciprocal_approx_*), gpsimd TT with broadcast APs or
comparison opcodes, EVENT_SEMAPHORE_RANGE_CLEAR.  Pool (gpsimd) is used
only for dense/strided add/mult/sub/copy.
"""
import sys

for _p in ("/opt/trn_rl_repo",):
    if _p not in sys.path:
        sys.path.insert(0, _p)

import numpy as np

import concourse.bass as bass
import concourse.tile as tile
from concourse import mybir
from concourse.bass_utils import run_bass_kernel_spmd

F32 = mybir.dt.float32
BF16 = mybir.dt.bfloat16
AX = mybir.AxisListType
OP = mybir.AluOpType
ACTF = mybir.ActivationFunctionType

B, P, C, K = 32, 8732, 81, 16
NCORES = 8
I = B // NCORES          # images per core = 4
PP = 8960                # padded priors = 70 * 128
T = PP // 128            # 70 prior tiles
T8 = 80                  # padded tile count for the m16 max tree
T2 = 72                  # padded tile count for 128-col transpose blocks
NB = T2 * K // 128       # 9 transpose blocks of 128 (t,k)-columns
NCH = 7                  # score chunks per image (10 tiles each)
CT = T // NCH            # tiles per chunk = 10
LN_THR = -1.0986122886681098   # ln(1/3); ov>=.5 <=> i/A>=1/3
KV0 = 104.0              # sentinel base: fmask*(104+0.5k) - 100 -> 4+0.5k
NQ = 5                   # gathered quantities per box (cx, cy, 5lnw, 5lnh, pad)

# f32 const blob column offsets
C_IO15 = 0
C_IV4 = C_IO15 + 16
C_PC4 = C_IV4 + T2 * 4
C_ONE = C_PC4 + T2 * 4
C_IDF = C_ONE + 1
C_O128 = C_IDF + 128
C_TOT = C_O128 + 128
# bf16 const blob column offsets (prior rows pre-expanded across K)
B_PX1 = 0
B_PY1 = B_PX1 + T * K
B_PX2 = B_PY1 + T * K
B_PY2 = B_PX2 + T * K
B_PAREA = B_PY2 + T * K
B_KV = B_PAREA + T * K
B_EPS = B_KV + T * K
B_TOT = B_EPS + 1

_bf16 = np.dtype("uint16")  # bf16 carried as uint16 bit pattern if ml_dtypes absent
try:
    import ml_dtypes

    _bf16 = np.dtype(ml_dtypes.bfloat16)
except ImportError:
    ml_dtypes = None


def _to_bf16(x: np.ndarray) -> np.ndarray:
    if ml_dtypes is not None:
        return x.astype(ml_dtypes.bfloat16)
    u = x.astype(np.float32).view(np.uint32)
    rounded = ((u >> 16) + ((u >> 15) & 1)).astype(np.uint32)
    return (rounded & 0xFFFF).astype(np.uint16)


def _fixup_module(nc: bass.Bass) -> None:
    """Adapt the Tile-generated module to this container's walrus build.

    - EVENT_SEMAPHORE_RANGE_CLEAR is rejected ("ISA wrong length"); the
      preceding Drain(is_reset_sema) already resets the same range, so drop it.
    - Seq-only instructions accept fewer sync waits than Tile emits; hoist
      excess waits onto NoOps placed immediately before (same engine, so
      program order preserves semantics).
    """
    import bass_rust

    for f in nc.m.functions:
        for blk in f.blocks:
            newl = []
            for ins in blk.instructions:
                if getattr(ins, "op_name", None) == "EVENT_SEMAPHORE_RANGE_CLEAR":
                    continue
                si = ins.sync_info
                maxw = 1
                if si is not None and si.on_wait and len(si.on_wait) > maxw:
                    waits = list(si.on_wait)
                    extra, keep = waits[:-maxw], waits[-maxw:]
                    for j in range(0, len(extra), 1):
                        nop = mybir.InstNoOp(
                            name=f"{ins.name}-wsplit{j}", ins=[], outs=[],
                            engine=ins.engine)
                        nop.sync_info = bass_rust.SyncInfo(
                            on_wait=[extra[j]], on_update=[])
                        newl.append(nop)
                    ins.sync_info = bass_rust.SyncInfo(
                        on_wait=keep,
                        on_update=list(si.on_update) if si.on_update else [])
                newl.append(ins)
            blk.instructions = newl


def build_nc(fixup: bool = True) -> bass.Bass:
    nc = bass.Bass()

    # p-major marshaled inputs: [*, 128, free] with contiguous per-partition rows
    d_scores = nc.dram_tensor("scores", [I, 128, T * C], BF16, kind="ExternalInput")
    d_locs = nc.dram_tensor("locs", [I, 128, T * 4], F32, kind="ExternalInput")
    d_cbf = nc.dram_tensor("cbf", [128, B_TOT], BF16, kind="ExternalInput")
    d_cf32 = nc.dram_tensor("cf32", [128, C_TOT], F32, kind="ExternalInput")
    # per-image free-layout box rows (bf16): 0 bx1 1 by1 2 bx2 3 by2 4 barea
    d_boxf = nc.dram_tensor("boxf", [I, 5, K], BF16, kind="ExternalInput")
    # per-image block-diagonal gather stationary [128, 8*NQ] (bf16)
    d_qblk = nc.dram_tensor("qblk", [I, 128, 8 * NQ], BF16, kind="ExternalInput")
    d_lblT = nc.dram_tensor("lblT", [I, K, 1], F32, kind="ExternalInput")
    # ln(parea + barea) per image, p-major bf16 (host-precomputed)
    d_lnA = nc.dram_tensor("lnA", [I, 128, T * K], BF16, kind="ExternalInput")
    d_ident = nc.dram_tensor("ident", [128, 128], BF16, kind="ExternalInput")
    d_ones_r = nc.dram_tensor("ones_r", [1, 128], F32, kind="ExternalInput")
    d_onesb = nc.dram_tensor("onesb", [1, 128], BF16, kind="ExternalInput")
    d_iota81 = nc.dram_tensor("iota81", [K, C], F32, kind="ExternalInput")
    # out row layout (single partition): [np0..3, box0..3, cep0..3, mine0..3]
    d_out = nc.dram_tensor("out", [1, 16], F32, kind="ExternalOutput")

    from contextlib import ExitStack

    with tile.TileContext(nc) as tc, ExitStack() as es:
        cpool = es.enter_context(tc.tile_pool(name="consts", bufs=1))
        spool = es.enter_context(tc.tile_pool(name="scores", bufs=4))
        prepool = es.enter_context(tc.tile_pool(name="prein", bufs=4))
        wpool = es.enter_context(tc.tile_pool(name="work", bufs=2))
        epool = es.enter_context(tc.tile_pool(name="exp", bufs=3))
        bpool = es.enter_context(tc.tile_pool(name="batched", bufs=1))
        pp_t = es.enter_context(tc.tile_pool(name="ps_t", bufs=1, space="PSUM"))
        pp_sel = es.enter_context(tc.tile_pool(name="ps_sel", bufs=1, space="PSUM"))
        pp_u = es.enter_context(tc.tile_pool(name="ps_u", bufs=1, space="PSUM"))
        pp_r = es.enter_context(tc.tile_pool(name="ps_r", bufs=2, space="PSUM"))
        rpool = es.enter_context(tc.tile_pool(name="redsb", bufs=4))

        # ------- constants + all per-image loads, DMA-issue spread over
        # ------- SP/ACT/Pool queues (engine-synchronous issue serializes)
        cbf = cpool.tile([128, B_TOT], BF16, tag="cbf")
        cf32 = cpool.tile([128, C_TOT], F32, tag="cf32")
        ident = cpool.tile([128, 128], BF16, tag="ident")
        ones_r = cpool.tile([1, 128], F32, tag="ones_r")
        onesb = cpool.tile([1, 128], BF16, tag="onesb")
        iota81 = cpool.tile([K, C], F32, tag="iota81")

        sres_l = [spool.tile([128, T, C], BF16, tag="sres", name=f"sres{i}")
                  for i in range(I)]
        l4_l = [prepool.tile([128, T2, 4], F32, tag="l4", name=f"l4{i}")
                for i in range(I)]
        boxf_l = [prepool.tile([1, 5, K], BF16, tag="boxf", name=f"boxf{i}")
                  for i in range(I)]
        qblk_l = [prepool.tile([128, 8 * NQ], BF16, tag="qblk",
                               name=f"qblk{i}") for i in range(I)]
        lbl_l = [prepool.tile([K, 1], F32, tag="lbl16", name=f"lbl{i}")
                 for i in range(I)]
        lnA_l = [prepool.tile([128, T, K], BF16, tag="lnA", name=f"lnA{i}")
                 for i in range(I)]

        # ACT queue: image-0 box row + onesb first (bb broadcast unblocks)
        nc.scalar.dma_start(out=boxf_l[0][:], in_=d_boxf[0, :, :][None])
        nc.scalar.dma_start(out=onesb[:], in_=d_onesb[:, :])
        for i in range(1, I):
            nc.scalar.dma_start(out=boxf_l[i][:], in_=d_boxf[i, :, :][None])
        nc.scalar.dma_start(out=sres_l[0][:].rearrange("p t c -> p (t c)"),
                            in_=d_scores[0, :, :])
        nc.scalar.dma_start(out=ident[:], in_=d_ident[:, :])
        nc.scalar.dma_start(out=iota81[:], in_=d_iota81[:, :])
        nc.scalar.dma_start(out=ones_r[:], in_=d_ones_r[:, :])
        # SP queue: jaccard consts, then remaining scores staged with the
        # small per-image tensors they unblock
        nc.sync.dma_start(out=cbf[:], in_=d_cbf[:, :])
        nc.sync.dma_start(out=lnA_l[0][:].rearrange("p t k -> p (t k)"),
                          in_=d_lnA[0, :, :])
        nc.sync.dma_start(out=sres_l[1][:].rearrange("p t c -> p (t c)"),
                          in_=d_scores[1, :, :])
        nc.sync.dma_start(out=cf32[:], in_=d_cf32[:, :])
        for i in range(1, I):
            nc.sync.dma_start(out=lnA_l[i][:].rearrange("p t k -> p (t k)"),
                              in_=d_lnA[i, :, :])
        for i in range(I):
            nc.vector.memset(l4_l[i][:, T:, :], 0.0)
        for i in range(I):
            nc.sync.dma_start(
                out=l4_l[i][:, :T, :].rearrange("p t d -> p (t d)"),
                in_=d_locs[i, :, :])
            nc.sync.dma_start(out=qblk_l[i][:], in_=d_qblk[i, :, :])
            nc.sync.dma_start(out=lbl_l[i][:], in_=d_lblT[i, :, :])
            if i >= 2:
                nc.sync.dma_start(
                    out=sres_l[i][:].rearrange("p t c -> p (t c)"),
                    in_=d_scores[i, :, :])

        def prow(off):  # bf16 pre-expanded prior row view [128, T, K]
            return cbf[:, off:off + T * K].rearrange("p (t k) -> p t k", k=K)

        pxe = {nm: prow(off) for nm, off in
               [("px1", B_PX1), ("py1", B_PY1), ("px2", B_PX2),
                ("py2", B_PY2), ("parea", B_PAREA)]}
        kvb = prow(B_KV)
        eps_b = cbf[:, B_EPS:B_EPS + 1]
        io15 = cf32[:, C_IO15:C_IV4]
        iv4 = cf32[:, C_IV4:C_PC4].rearrange("p (t d) -> p t d", d=4)
        pc4 = cf32[:, C_PC4:C_ONE].rearrange("p (t d) -> p t d", d=4)
        ones_p = cf32[:, C_ONE:C_IDF]
        identf = cf32[:, C_IDF:C_O128]
        ones128 = cf32[:, C_O128:C_TOT]

        def rowsum(dst_row_ap, src_ap, n):
            """[P, n] f32 -> [1, n] partition sum written to dst_row_ap."""
            ps = pp_r.tile([128, 128], F32, tag="red_bc")
            nc.tensor.matmul(ps[0:1, :n], lhsT=ones_p[:src_ap.shape[0], :],
                             rhs=src_ap, start=True, stop=True)
            nc.vector.tensor_copy(dst_row_ap, ps[0:1, :n])

        def bcast_row(dst_ap, row_ap, n):
            """[1, n] f32 -> [128, n] replicated (dst may be bf16)."""
            ps = pp_r.tile([128, 128], F32, tag="red_bc")
            nc.tensor.matmul(ps[:, :n], lhsT=ones_r[:], rhs=row_ap,
                             start=True, stop=True)
            nc.vector.tensor_copy(dst_ap, ps[:, :n])

        def allreduce_sum(dst_ap, src_ap, n):
            ps = pp_r.tile([128, 128], F32, tag="red_bc")
            nc.tensor.matmul(ps[:, :n], lhsT=ones128, rhs=src_ap,
                             start=True, stop=True)
            nc.vector.tensor_copy(dst_ap, ps[:, :n])

        def maxreduce_row(dst_row_ap, src_ap, n):
            """[128, n] f32 -> [1, n] partition max written to dst_row_ap."""
            ps = pp_r.tile([128, 128], F32, tag="red_bc")
            nc.tensor.transpose(ps[:n, :], src_ap, identf)
            tsb = rpool.tile([128, 128], F32, tag="red_tsb")
            nc.vector.tensor_copy(tsb[:n, :], ps[:n, :])
            mx = rpool.tile([128, 1], F32, tag="red_mx")
            nc.vector.tensor_reduce(out=mx[:n, :], in_=tsb[:n, :],
                                    axis=AX.X, op=OP.max)
            ps2 = pp_r.tile([128, 128], F32, tag="red_bc")
            nc.tensor.transpose(ps2[0:1, :n], mx[:n, :], identf[:n, :n])
            nc.vector.tensor_copy(dst_row_ap, ps2[0:1, :n])

        # batched buffers [128, I, T]
        se4 = bpool.tile([128, I, T], BF16, tag="se4")
        lse4 = bpool.tile([128, I, T], F32, tag="lse4")
        cen4 = bpool.tile([128, I, T], BF16, tag="cen4")
        np4 = bpool.tile([128, I], F32, tag="np4")
        k34 = bpool.tile([128, I], F32, tag="k34")
        cnt = bpool.tile([128, I, 16], F32, tag="cnt")
        cntr = bpool.tile([128, I, 16], F32, tag="cntr")
        lo4 = bpool.tile([128, I], F32, tag="lo4")
        lop = bpool.tile([128, I], F32, tag="lop")
        hi4 = bpool.tile([128, I], F32, tag="hi4")
        scadd = bpool.tile([128, I, 4], F32, tag="scadd")   # fs, cn, lps, box
        scrow = bpool.tile([1, I, 4], F32, tag="scrow")
        bm4 = bpool.tile([128, I], F32, tag="bm4")
        bmrow = bpool.tile([1, I], F32, tag="bmrow")
        uf4 = bpool.tile([1, I], F32, tag="uf4")
        out_sb = bpool.tile([1, 16], F32, tag="out_sb")

        def emit_mining(i):
            # ---------------- mining (per image, pipelined) --------------
            nc.vector.tensor_scalar(out=k34[:, i:i + 1], in0=np4[:, i:i + 1],
                                    scalar1=3.0, scalar2=None, op0=OP.mult)
            allreduce_sum(cntr[:, i, :], cnt[:, i, :], 16)
            # lo = (#edges with count >= k) - 1   (edges j = 0..15)
            ge16 = bpool.tile([128, 16], F32, tag="ge16")
            nc.vector.tensor_scalar(out=ge16[:], in0=cntr[:, i, :],
                                    scalar1=k34[:, i:i + 1], scalar2=None,
                                    op0=OP.is_ge)
            nc.vector.tensor_reduce(out=lo4[:, i:i + 1], in_=ge16[:],
                                    axis=AX.X, op=OP.add)
            nc.vector.tensor_scalar(out=lo4[:, i:i + 1], in0=lo4[:, i:i + 1],
                                    scalar1=-1.0, scalar2=None, op0=OP.add)
            nc.vector.tensor_scalar(out=lop[:, i:i + 1], in0=lo4[:, i:i + 1],
                                    scalar1=1.0 / 16, scalar2=None, op0=OP.add)
            # level 2: thresholds lo + m/16 (io15 has (1..15)/16 then +999)
            thr2 = bpool.tile([128, 16], F32, tag="thr2")
            nc.vector.tensor_scalar(out=thr2[:], in0=io15,
                                    scalar1=lo4[:, i:i + 1], scalar2=None,
                                    op0=OP.add)
            c2 = bpool.tile([128, 16], F32, tag="c2")
            msci2 = wpool.tile([128, T], BF16, tag="msci2")
            for m in range(16):
                nc.vector.tensor_scalar(out=msci2[:], in0=cen4[:, i, :],
                                        scalar1=thr2[:, m:m + 1], scalar2=None,
                                        op0=OP.is_gt, op1=OP.add,
                                        accum_out=c2[:, m:m + 1])
            c2r = bpool.tile([128, 16], F32, tag="c2r")
            allreduce_sum(c2r[:], c2[:], 16)
            mc = bpool.tile([128, 1], F32, tag="mc")
            nc.vector.tensor_scalar(out=ge16[:], in0=c2r[:],
                                    scalar1=k34[:, i:i + 1], scalar2=None,
                                    op0=OP.is_ge, op1=OP.add, accum_out=mc[:])
            nc.vector.tensor_scalar(out=hi4[:, i:i + 1], in0=mc[:],
                                    scalar1=1.0 / 16, scalar2=lop[:, i:i + 1],
                                    op0=OP.mult, op1=OP.add)
            # F(hi), count(hi), boundary max
            fsc = bpool.tile([128, T], BF16, tag="fsc")
            nc.vector.scalar_tensor_tensor(
                out=fsc[:], in0=cen4[:, i, :], scalar=hi4[:, i:i + 1],
                in1=cen4[:, i, :], op0=OP.is_gt, op1=OP.mult,
                accum_out=scadd[:, i, 0:1])
            nc.vector.tensor_scalar(out=fsc[:], in0=cen4[:, i, :],
                                    scalar1=hi4[:, i:i + 1], scalar2=None,
                                    op0=OP.is_gt, op1=OP.add,
                                    accum_out=scadd[:, i, 1:2])
            nc.vector.scalar_tensor_tensor(
                out=fsc[:], in0=cen4[:, i, :], scalar=hi4[:, i:i + 1],
                in1=cen4[:, i, :], op0=OP.is_le, op1=OP.mult)
            nc.vector.tensor_reduce(out=bm4[:, i:i + 1], in_=fsc[:],
                                    axis=AX.X, op=OP.max)

        bb_l = []
        for i in range(I):
            bbt = prepool.tile([128, 5, K], BF16, tag="bb", name=f"bb{i}")
            ps_bb = pp_r.tile([128, 128], F32, tag="red_bc",
                              name=f"psbb{i}")
            nc.tensor.matmul(ps_bb[:, :5 * K], lhsT=onesb[:],
                             rhs=boxf_l[i][:].rearrange("p a k -> p (a k)"),
                             start=True, stop=True)
            nc.vector.tensor_copy(bbt[:].rearrange("p a k -> p (a k)"),
                                  ps_bb[:, :5 * K])
            bb_l.append(bbt)

        for i in range(I):
            sres = sres_l[i]
            l4 = l4_l[i]
            qblk = qblk_l[i]
            lbl16 = lbl_l[i]
            bb = bb_l[i]

            def bcast_b(row):  # [128, K] box row -> [128, T, K] AP (packed k)
                return bb[:, row, :][:, None, :].broadcast_to([128, T, K])

            # ---------------- jaccard (log domain, bf16) -----------------
            # ordering uses i/(pa+ba): x/(A-x) is a monotone bijection of
            # x/A, so argmax/threshold semantics match IoU with thr ln(1/3)
            lov80 = wpool.tile([128, T8, K], BF16, tag="lov80")
            nc.vector.memset(lov80[:, T:, :], -100.0)
            lov = lov80[:, :T, :]
            ltx = wpool.tile([128, T, K], BF16, tag="ltx")
            lty = wpool.tile([128, T, K], BF16, tag="lty")
            w0 = wpool.tile([128, T, K], BF16, tag="w0")
            h0 = wpool.tile([128, T, K], BF16, tag="h0")
            wr = wpool.tile([128, T, K], BF16, tag="wr")
            hr = wpool.tile([128, T, K], BF16, tag="hr")
            inter = wpool.tile([128, T, K], BF16, tag="inter")

            nc.vector.tensor_tensor(out=ltx[:], in0=pxe["px1"],
                                    in1=bcast_b(0), op=OP.max)
            nc.vector.tensor_tensor(out=lty[:], in0=pxe["py1"],
                                    in1=bcast_b(1), op=OP.max)
            nc.vector.tensor_tensor(out=w0[:], in0=pxe["px2"],
                                    in1=bcast_b(2), op=OP.min)
            nc.vector.tensor_tensor(out=h0[:], in0=pxe["py2"],
                                    in1=bcast_b(3), op=OP.min)
            nc.vector.tensor_sub(wr[:], w0[:], ltx[:])
            nc.vector.tensor_sub(hr[:], h0[:], lty[:])
            nc.scalar.activation(wr[:], wr[:], ACTF.Relu)
            nc.scalar.activation(hr[:], hr[:], ACTF.Relu)
            nc.vector.tensor_mul(inter[:], wr[:], hr[:])
            nc.scalar.activation(inter[:], inter[:], ACTF.Ln, bias=eps_b)
            nc.vector.tensor_sub(lov, inter[:], lnA_l[i][:])

            # ---------------- matching pass 2 ----------------
            # per-box max over priors: dense max tree (80 = 2*2*2*2*5)
            tm1 = wpool.tile([128, 40, K], BF16, tag="tm1")
            nc.vector.tensor_tensor(out=tm1[:], in0=lov80[:, :40, :],
                                    in1=lov80[:, 40:, :], op=OP.max)
            tm2 = wpool.tile([128, 20, K], BF16, tag="tm2")
            nc.vector.tensor_tensor(out=tm2[:], in0=tm1[:, :20, :],
                                    in1=tm1[:, 20:, :], op=OP.max)
            tm3 = wpool.tile([128, 10, K], BF16, tag="tm3")
            nc.vector.tensor_tensor(out=tm3[:], in0=tm2[:, :10, :],
                                    in1=tm2[:, 10:, :], op=OP.max)
            tm4 = wpool.tile([128, 5, K], BF16, tag="tm4")
            nc.vector.tensor_tensor(out=tm4[:], in0=tm3[:, :5, :],
                                    in1=tm3[:, 5:, :], op=OP.max)
            m16 = wpool.tile([128, K], F32, tag="m16")
            nc.vector.tensor_reduce(
                out=m16[:], in_=tm4[:].rearrange("p t k -> p k t"),
                axis=AX.X, op=OP.max)
            m16row = wpool.tile([1, K], F32, tag="m16row")
            maxreduce_row(m16row[:], m16[:], K)
            m16rb = wpool.tile([128, K], BF16, tag="m16rb")
            bcast_row(m16rb[:], m16row[:], K)
            fmask = wpool.tile([128, T, K], BF16, tag="fmask")
            nc.vector.tensor_tensor(
                out=fmask[:], in0=lov,
                in1=m16rb[:][:, None, :].broadcast_to([128, T, K]),
                op=OP.is_equal)
            ovf = wpool.tile([128, T, K], BF16, tag="ovf")
            fm2 = wpool.tile([128, T, K], BF16, tag="fm2")
            nc.vector.tensor_mul(fm2[:], fmask[:], kvb)
            nc.vector.scalar_tensor_tensor(
                out=ovf[:], in0=fm2[:], scalar=-100.0, in1=lov,
                op0=OP.add, op1=OP.max)
            # per-prior max over k: dense tree on the packed innermost dim
            ms1 = wpool.tile([128, T, 8], BF16, tag="ms1")
            nc.vector.tensor_tensor(out=ms1[:], in0=ovf[:, :, 0:8],
                                    in1=ovf[:, :, 8:16], op=OP.max)
            ms2 = wpool.tile([128, T, 4], BF16, tag="ms2")
            nc.vector.tensor_tensor(out=ms2[:], in0=ms1[:, :, 0:4],
                                    in1=ms1[:, :, 4:8], op=OP.max)
            ms3 = wpool.tile([128, T, 2], BF16, tag="ms3")
            nc.vector.tensor_tensor(out=ms3[:], in0=ms2[:, :, 0:2],
                                    in1=ms2[:, :, 2:4], op=OP.max)
            pm = wpool.tile([128, T], BF16, tag="pm")
            nc.vector.tensor_tensor(out=pm[:], in0=ms3[:, :, 0],
                                    in1=ms3[:, :, 1], op=OP.max)
            # pmz = pm where positive else pm+1 (matches nothing): fuses the
            # one-hot and the pos mask into a single is_eq
            pmz = wpool.tile([128, T], BF16, tag="pmz")
            nc.vector.scalar_tensor_tensor(
                out=pmz[:], in0=pm[:], scalar=LN_THR, in1=pm[:],
                op0=OP.is_lt, op1=OP.add)
            wm72 = wpool.tile([128, T2 * K], BF16, tag="wm72")
            nc.vector.memset(wm72[:, T * K:], 0.0)
            wmat = wm72[:, :T * K].rearrange("p (t k) -> p t k", k=K)
            nc.vector.tensor_tensor(
                out=wmat, in0=ovf[:],
                in1=pmz[:][:, :, None].broadcast_to([128, T, K]),
                op=OP.is_equal)
            pos72 = wpool.tile([128, T2], F32, tag="pos72")
            nc.vector.memset(pos72[:, T:], 0.0)
            npt = wpool.tile([128, 1], F32, tag="npt")
            nc.vector.tensor_scalar(out=pos72[:, :T], in0=pm[:],
                                    scalar1=LN_THR, scalar2=None,
                                    op0=OP.is_ge, op1=OP.add, accum_out=npt[:])
            allreduce_sum(np4[:, i:i + 1], npt[:], 1)

            if i > 0:
                emit_mining(i - 1)

            # ---------------- box gather via PE ----------------
            ohT_ps = pp_t.tile([128, NB, 128], BF16, tag="ohT")
            for b in range(NB):
                nc.tensor.transpose(
                    ohT_ps[:, b, :],
                    wm72[:, b * 128:(b + 1) * 128],
                    ident[:])
            ohT_sb = wpool.tile([128, NB * 128], BF16, tag="ohT_sb")
            nc.scalar.copy(ohT_sb[:], ohT_ps[:].rearrange("p b n -> p (b n)"))

            sel_ps = pp_sel.tile([8 * NQ, NB, 128], F32, tag="sel")
            for b in range(NB):
                nc.tensor.matmul(sel_ps[:, b, :], lhsT=qblk[:],
                                 rhs=ohT_sb[:, b * 128:(b + 1) * 128],
                                 start=True, stop=True)
            sel_sb = wpool.tile([8 * NQ, NB * 128], BF16, tag="sel_sb")
            nc.scalar.copy(sel_sb[:], sel_ps[:].rearrange("p b n -> p (b n)"))
            bk_ps = pp_t.tile([128, NB, 8 * NQ], BF16, tag="ohT")
            for b in range(NB):
                nc.tensor.transpose(
                    bk_ps[:, b, :],
                    sel_sb[:, b * 128:(b + 1) * 128],
                    ident[:8 * NQ, :8 * NQ])
            selq = wpool.tile([128, NB * 8 * NQ], BF16, tag="selq")
            nc.scalar.copy(selq[:], bk_ps[:].rearrange("p b n -> p (b n)"))
            # selq[p, (blk*40 + tb*5 + q)] = sel_q at t = blk*8+tb
            sel4 = selq[:].rearrange("p (t q) -> p t q", q=NQ)[:, :, 0:4]

            # ---------------- box L1 (Pool chain + ACT abs-accum) ---------
            lp4 = wpool.tile([128, T2, 4], F32, tag="lp4")
            nc.gpsimd.tensor_add(lp4[:], l4[:], pc4)
            tb1 = wpool.tile([128, T2, 4], F32, tag="tb1")
            nc.gpsimd.tensor_mul(tb1[:], sel4, iv4)
            nc.gpsimd.tensor_sub(tb1[:], lp4[:], tb1[:])
            nc.vector.tensor_tensor(
                out=tb1[:], in0=tb1[:],
                in1=pos72[:][:, :, None].broadcast_to([128, T2, 4]),
                op=OP.mult)
            nc.scalar.activation(tb1[:], tb1[:], ACTF.Abs,
                                 accum_out=scadd[:, i, 3:4])

            # ---------------- U matrix (score at label) ----------------
            u_ps = pp_u.tile([K, C], F32, tag="u")
            for t_ in range(T):
                nc.tensor.matmul(u_ps[:], lhsT=wmat[:, t_, :],
                                 rhs=sres[:, t_, :],
                                 start=(t_ == 0), stop=(t_ == T - 1))
            u_sb = wpool.tile([K, C], F32, tag="u_sb")
            nc.scalar.copy(u_sb[:], u_ps[:])
            ufx = wpool.tile([K, C], F32, tag="ufx")
            ufa = wpool.tile([K, 1], F32, tag="ufa")
            nc.vector.scalar_tensor_tensor(
                out=ufx[:], in0=iota81[:], scalar=lbl16[:], in1=u_sb[:],
                op0=OP.is_equal, op1=OP.mult, accum_out=ufa[:])
            rowsum(uf4[:, i:i + 1], ufa[:], 1)

            # ------- CE: exp (ACT) + class sums (Pool half-adds + DVE) ----
            for ch in range(NCH):
                et = epool.tile([128, CT, C], BF16, tag="exps")
                nc.scalar.activation(
                    et[:], sres[:, ch * CT:(ch + 1) * CT, :], ACTF.Exp)
                et2 = epool.tile([128, CT, 40], BF16, tag="et2")
                sl = se4[:, i, ch * CT:(ch + 1) * CT]
                with nc.allow_low_precision("bf16 class sums"):
                    nc.vector.tensor_add(et2[:], et[:, :, 0:40],
                                         et[:, :, 40:80])
                    nc.vector.tensor_reduce(out=sl, in_=et2[:], axis=AX.X,
                                            op=OP.add)
                    nc.vector.tensor_add(sl, sl, et[:, :, 80])
            # lse per image, ce0, cen = (1-pos)*ce0 fused
            nc.scalar.activation(lse4[:, i, :], se4[:, i, :], ACTF.Ln)
            ce0 = wpool.tile([128, T], F32, tag="ce0")
            nc.vector.tensor_sub(ce0[:], lse4[:, i, :], sres[:, :, 0])
            nc.vector.scalar_tensor_tensor(
                out=cen4[:, i, :], in0=pos72[:, :T], scalar=0.5,
                in1=ce0[:], op0=OP.is_lt, op1=OP.mult)
            msci = wpool.tile([128, T], BF16, tag="msci")
            for j in range(16):
                nc.vector.tensor_scalar(out=msci[:], in0=cen4[:, i, :],
                                        scalar1=float(j), scalar2=None,
                                        op0=OP.is_gt, op1=OP.add,
                                        accum_out=cnt[:, i, j:j + 1])
            lpst = wpool.tile([128, T], F32, tag="lpst")
            nc.vector.scalar_tensor_tensor(
                out=lpst[:], in0=pos72[:, :T], scalar=1.0,
                in1=lse4[:, i, :], op0=OP.mult, op1=OP.mult,
                accum_out=scadd[:, i, 2:3])

        emit_mining(I - 1)

        rowsum(scrow[:].rearrange("p i s -> p (i s)"),
               scadd[:].rearrange("p i s -> p (i s)"), I * 4)
        maxreduce_row(bmrow[:], bm4[:], I)

        # ---------------- final combine (partition 0) ----------------
        r4 = bpool.tile([1, I], F32, tag="r4")
        nc.vector.tensor_sub(r4[:], k34[0:1, :], scrow[:, :, 1])
        nc.vector.tensor_mul(r4[:], r4[:], bmrow[:])
        nc.vector.tensor_add(r4[:], r4[:], scrow[:, :, 0])   # mine sums
        cep = bpool.tile([1, I], F32, tag="cep")
        nc.vector.tensor_sub(cep[:], scrow[:, :, 2], uf4[:])  # ce_pos sums
        nc.vector.tensor_copy(out_sb[:, 0:4], np4[0:1, :])
        nc.vector.tensor_copy(out_sb[:, 4:8], scrow[:, :, 3])
        nc.vector.tensor_copy(out_sb[:, 8:12], cep[:])
        nc.vector.tensor_copy(out_sb[:, 12:16], r4[:])
        nc.sync.dma_start(out=d_out[:, :], in_=out_sb[:])

    if fixup:
        _fixup_module(nc)
    return nc


def prepare_inputs(predicted_locs, predicted_scores, boxes, labels,
                   priors_centers):
    """Shard + marshal the full inputs into 8 per-core in_maps (p-major)."""
    predicted_locs = np.asarray(predicted_locs, np.float32)
    predicted_scores = np.asarray(predicted_scores, np.float32)
    boxes = np.asarray(boxes, np.float32)
    labels_f = np.asarray(labels).astype(np.float32)
    priors = np.asarray(priors_centers, np.float32)

    npad = PP - P
    # scores: pad rows have class0=0, others -50 -> lse=0, S0=0, ce0=0 exactly
    pad_scores = np.full((B, npad, C), -50.0, np.float32)
    pad_scores[:, :, 0] = 0.0
    scores_p = np.concatenate([predicted_scores, pad_scores], axis=1)
    # p-major: [B, 128, T*C]
    scores_pm = np.ascontiguousarray(
        scores_p.reshape(B, T, 128, C).transpose(0, 2, 1, 3)
    ).reshape(B, 128, T * C)
    scores_bf = _to_bf16(scores_pm)
    locs_p = np.concatenate(
        [predicted_locs, np.zeros((B, npad, 4), np.float32)], axis=1)
    locs_pm = np.ascontiguousarray(
        locs_p.reshape(B, T, 128, 4).transpose(0, 2, 1, 3)
    ).reshape(B, 128, T * 4)

    # prior rows pre-expanded across K (p-major, bf16)
    pad_pri = np.tile(np.array([-100.0, -100.0, 1.0, 1.0], np.float32),
                      (npad, 1))
    pri = np.concatenate([priors, pad_pri], axis=0)
    pcx, pcy, pw, ph = pri[:, 0], pri[:, 1], pri[:, 2], pri[:, 3]

    def pm_grid(v):  # [PP] -> [128, T]
        return np.ascontiguousarray(v.astype(np.float32).reshape(T, 128).T)

    def pexp(v):  # [PP] -> [128, T*K] expanded across K
        return np.repeat(pm_grid(v)[:, :, None], K, axis=2).reshape(128, T * K)

    kv = np.tile(KV0 + 0.5 * np.arange(K, dtype=np.float32), (128, T))
    eps_b = np.full((128, 1), 1e-20, np.float32)
    cbf = _to_bf16(np.concatenate(
        [pexp(pcx - pw / 2), pexp(pcy - ph / 2), pexp(pcx + pw / 2),
         pexp(pcy + ph / 2), pexp(pw * ph), kv, eps_b], axis=1))
    assert cbf.shape[1] == B_TOT

    # iv4/pc4 [128, T2, 4] host-assembled (d = x, y, w, h; tail zero)
    iv4 = np.zeros((128, T2, 4), np.float32)
    pc4 = np.zeros((128, T2, 4), np.float32)
    iv4[:, :T, 0] = pm_grid(10.0 / pw)
    iv4[:, :T, 1] = pm_grid(10.0 / ph)
    iv4[:, :T, 2] = 1.0
    iv4[:, :T, 3] = 1.0
    pc4[:, :T, 0] = pm_grid(pcx * (10.0 / pw))
    pc4[:, :T, 1] = pm_grid(pcy * (10.0 / ph))
    pc4[:, :T, 2] = pm_grid(5.0 * np.log(pw))
    pc4[:, :T, 3] = pm_grid(5.0 * np.log(ph))

    io15 = np.tile(np.concatenate([np.arange(1, 16, dtype=np.float32) / 16.0,
                                   [999.0]]), (128, 1))
    ones_p = np.ones((128, 1), np.float32)
    identf = np.eye(128, dtype=np.float32)
    cf32 = np.concatenate(
        [io15, iv4.reshape(128, T2 * 4), pc4.reshape(128, T2 * 4),
         ones_p, identf, np.ones((128, 128), np.float32)],
        axis=1).astype(np.float32)
    assert cf32.shape[1] == C_TOT

    bx1, by1, bx2, by2 = (boxes[:, :, d] for d in range(4))
    barea = (bx2 - bx1) * (by2 - by1)
    boxf = _to_bf16(np.stack([bx1, by1, bx2, by2, barea], axis=1))
    # lnA[b, p, t, k] = ln(parea[p,t] + barea[b,k]) (p-major)
    parea_pm = pm_grid(pw * ph)                               # [128, T]
    lnA = _to_bf16(np.log(
        parea_pm[None, :, :, None] + barea[:, None, None, :].astype(np.float64)
    ).reshape(B, 128, T * K))
    q5 = np.stack([
        (bx1 + bx2) / 2, (by1 + by2) / 2,
        5.0 * np.log(bx2 - bx1), 5.0 * np.log(by2 - by1),
        np.zeros_like(bx1),
    ], axis=2).astype(np.float32)                           # [B, K, 5]
    qblk = np.zeros((B, 128, 8 * NQ), np.float32)
    for tb in range(8):
        qblk[:, tb * K:(tb + 1) * K, tb * NQ:(tb + 1) * NQ] = q5
    qblk = _to_bf16(qblk)

    ident = _to_bf16(np.eye(128, dtype=np.float32))
    ones_r = np.ones((1, 128), np.float32)
    onesb = _to_bf16(np.ones((1, 128), np.float32))
    iota81 = np.tile(np.arange(C, dtype=np.float32), (K, 1))

    in_maps = []
    for c in range(NCORES):
        sl = slice(c * I, (c + 1) * I)
        in_maps.append({
            "scores": scores_bf[sl],
            "locs": locs_pm[sl],
            "cbf": cbf,
            "cf32": cf32,
            "boxf": boxf[sl],
            "qblk": qblk[sl],
            "lblT": labels_f[sl][:, :, None],
            "lnA": lnA[sl],
            "ident": ident,
            "ones_r": ones_r,
            "onesb": onesb,
            "iota81": iota81,
        })
    return in_maps


def combine_outputs(outs):
    """outs: list of 8 per-core [1,16] arrays -> scalar loss."""
    parts = np.concatenate([o.reshape(4, 4) for o in outs], axis=1)  # [4, 32]
    n_pos_total = parts[0].sum()
    box_sum = parts[1].sum()
    class_sum = parts[2].sum() + parts[3].sum()
    loss = class_sum / n_pos_total + box_sum / (n_pos_total * 4.0)
    return np.float32(loss)


_NC_CACHE = {}


def kernel(predicted_locs, predicted_scores, boxes, labels, priors_centers):
    if "nc" not in _NC_CACHE:
        _NC_CACHE["nc"] = build_nc()
    nc = _NC_CACHE["nc"]
    in_maps = prepare_inputs(predicted_locs, predicted_scores, boxes, labels,
                             priors_centers)
    res = run_bass_kernel_spmd(nc, in_maps, list(range(NCORES)))
    outs = [res.results[c]["out"] for c in range(NCORES)]
    return combine_outputs(outs)


if __name__ == "__main__":
    import reference as R

    inputs = {k: np.asarray(v) for k, v in R.setup_inputs().items()}
    print("loss =", kernel(**inputs))


# revision 46
# speedup vs baseline: 1.1137x; 1.0080x over previous
"""Trainium2 Bass kernel for SSD MultiBox loss (nn_ModelLoss_5970004541458).

Strategy: data-parallel over batch (32 images -> 8 cores x 4 images).
Per core, everything over the prior dim (P=8732, padded to 8960 = 70*128)
runs on-device:
  - jaccard matching (16 boxes x 8960 priors) in bf16 in the LOG domain:
    overlap values only feed max/argmax/threshold comparisons, and ln is
    strictly monotone, so lov = Ln(inter+eps) - Ln(den) (both Lns on ACT)
    replaces the expensive DVE reciprocal.  Prior rows are pre-expanded
    across K on the host so every DVE tensor op has packed 2-byte operands
    (2x/4x mode).  Forced-assignment sentinels: fmask*(104+0.5k) - 100 =
    4+0.5k where forced (bf16-exact steps), -100 (below any lov) where not.
  - per-prior one-hot box gather via PE transpose + block-diag matmul (bf16)
  - CE: exp on ACT (bf16); class-sums via Pool half-add + short DVE reduce
  - hard-negative mining via a 2-level 16-way counting grid with bounded-
    error boundary correction (no sort), cen carried in bf16
All inputs are marshaled host-side into p-major layouts so every DMA is
contiguous per partition (128 large descriptors, not 8960 tiny ones).
Each core returns 16 partial sums; the host combines them into the loss.

This walrus build rejects: gpsimd partition_all_reduce/partition_broadcast,
custom-DVE ops (reciprocal_approx_*), gpsimd TT with broadcast APs or
comparison opcodes, EVENT_SEMAPHORE_RANGE_CLEAR.  Pool (gpsimd) is used
only for dense/strided add/mult/sub/copy.
"""
import sys

for _p in ("/opt/trn_rl_repo",):
    if _p not in sys.path:
        sys.path.insert(0, _p)

import numpy as np

import concourse.bass as bass
import concourse.tile as tile
from concourse import mybir
from concourse.bass_utils import run_bass_kernel_spmd

F32 = mybir.dt.float32
BF16 = mybir.dt.bfloat16
AX = mybir.AxisListType
OP = mybir.AluOpType
ACTF = mybir.ActivationFunctionType

B, P, C, K = 32, 8732, 81, 16
NCORES = 8
I = B // NCORES          # images per core = 4
PP = 8960                # padded priors = 70 * 128
T = PP // 128            # 70 prior tiles
T8 = 80                  # padded tile count for the m16 max tree
T2 = 72                  # padded tile count for 128-col transpose blocks
NB = T2 * K // 128       # 9 transpose blocks of 128 (t,k)-columns
NCH = 7                  # score chunks per image (10 tiles each)
CT = T // NCH            # tiles per chunk = 10
LN_THR = -1.0986122886681098   # ln(1/3); ov>=.5 <=> i/A>=1/3
KV0 = 104.0              # sentinel base: fmask*(104+0.5k) - 100 -> 4+0.5k
NQ = 5                   # gathered quantities per box (cx, cy, 5lnw, 5lnh, pad)

# f32 const blob column offsets
C_IO15 = 0
C_IV4 = C_IO15 + 16
C_PC4 = C_IV4 + T2 * 4
C_ONE = C_PC4 + T2 * 4
C_IDF = C_ONE + 1
C_O128 = C_IDF + 128
C_TOT = C_O128 + 128
# bf16 const blob column offsets (prior rows pre-expanded across K)
B_PX1 = 0
B_PY1 = B_PX1 + T * K
B_PX2 = B_PY1 + T * K
B_PY2 = B_PX2 + T * K
B_PAREA = B_PY2 + T * K
B_KV = B_PAREA + T * K
B_EPS = B_KV + T * K
B_TOT = B_EPS + 1

_bf16 = np.dtype("uint16")  # bf16 carried as uint16 bit pattern if ml_dtypes absent
try:
    import ml_dtypes

    _bf16 = np.dtype(ml_dtypes.bfloat16)
except ImportError:
    ml_dtypes = None


def _to_bf16(x: np.ndarray) -> np.ndarray:
    if ml_dtypes is not None:
        return x.astype(ml_dtypes.bfloat16)
    u = x.astype(np.float32).view(np.uint32)
    rounded = ((u >> 16) + ((u >> 15) & 1)).astype(np.uint32)
    return (rounded & 0xFFFF).astype(np.uint16)


def _fixup_module(nc: bass.Bass) -> None:
    """Adapt the Tile-generated module to this container's walrus build.

    - EVENT_SEMAPHORE_RANGE_CLEAR is rejected ("ISA wrong length"); the
      preceding Drain(is_reset_sema) already resets the same range, so drop it.
    - Seq-only instructions accept fewer sync waits than Tile emits; hoist
      excess waits onto NoOps placed immediately before (same engine, so
      program order preserves semantics).
    """
    import bass_rust

    for f in nc.m.functions:
        for blk in f.blocks:
            newl = []
            for ins in blk.instructions:
                if getattr(ins, "op_name", None) == "EVENT_SEMAPHORE_RANGE_CLEAR":
                    continue
                si = ins.sync_info
                maxw = 1
                if si is not None and si.on_wait and len(si.on_wait) > maxw:
                    waits = list(si.on_wait)
                    extra, keep = waits[:-maxw], waits[-maxw:]
                    for j in range(0, len(extra), 1):
                        nop = mybir.InstNoOp(
                            name=f"{ins.name}-wsplit{j}", ins=[], outs=[],
                            engine=ins.engine)
                        nop.sync_info = bass_rust.SyncInfo(
                            on_wait=[extra[j]], on_update=[])
                        newl.append(nop)
                    ins.sync_info = bass_rust.SyncInfo(
                        on_wait=keep,
                        on_update=list(si.on_update) if si.on_update else [])
                newl.append(ins)
            blk.instructions = newl


def build_nc(fixup: bool = True) -> bass.Bass:
    nc = bass.Bass()

    # p-major marshaled inputs: [*, 128, free] with contiguous per-partition rows
    d_scores = nc.dram_tensor("scores", [I, 128, T * C], BF16, kind="ExternalInput")
    d_locs = nc.dram_tensor("locs", [128, I * T * 4], F32, kind="ExternalInput")
    d_cbf = nc.dram_tensor("cbf", [128, B_TOT], BF16, kind="ExternalInput")
    d_cf32 = nc.dram_tensor("cf32", [128, C_TOT], F32, kind="ExternalInput")
    # all images' box rows (5K each) + onesb row, single partition row
    d_boxall = nc.dram_tensor("boxall", [1, I * 5 * K + 128], BF16,
                              kind="ExternalInput")
    # gather stationaries for all images, p-major
    d_qall = nc.dram_tensor("qall", [128, I * 8 * NQ], BF16,
                            kind="ExternalInput")
    d_lblall = nc.dram_tensor("lblall", [K, I], F32, kind="ExternalInput")
    # ln(parea + barea) per image, p-major bf16 (host-precomputed)
    d_lnA = nc.dram_tensor("lnA", [I, 128, T * K], BF16, kind="ExternalInput")
    d_ident = nc.dram_tensor("ident", [128, 128], BF16, kind="ExternalInput")
    d_ones_r = nc.dram_tensor("ones_r", [1, 128], F32, kind="ExternalInput")
    d_iota81 = nc.dram_tensor("iota81", [K, C], F32, kind="ExternalInput")
    # out row layout (single partition): [np0..3, box0..3, cep0..3, mine0..3]
    d_out = nc.dram_tensor("out", [1, 16], F32, kind="ExternalOutput")

    from contextlib import ExitStack

    with tile.TileContext(nc) as tc, ExitStack() as es:
        cpool = es.enter_context(tc.tile_pool(name="consts", bufs=1))
        spool = es.enter_context(tc.tile_pool(name="scores", bufs=4))
        prepool = es.enter_context(tc.tile_pool(name="prein", bufs=4))
        wpool = es.enter_context(tc.tile_pool(name="work", bufs=2))
        epool = es.enter_context(tc.tile_pool(name="exp", bufs=4))
        bpool = es.enter_context(tc.tile_pool(name="batched", bufs=1))
        pp_t = es.enter_context(tc.tile_pool(name="ps_t", bufs=1, space="PSUM"))
        pp_sel = es.enter_context(tc.tile_pool(name="ps_sel", bufs=1, space="PSUM"))
        pp_u = es.enter_context(tc.tile_pool(name="ps_u", bufs=1, space="PSUM"))
        pp_r = es.enter_context(tc.tile_pool(name="ps_r", bufs=2, space="PSUM"))
        rpool = es.enter_context(tc.tile_pool(name="redsb", bufs=4))

        # ------- constants + all per-image loads, DMA-issue spread over
        # ------- SP/ACT/Pool queues (engine-synchronous issue serializes)
        cbf = cpool.tile([128, B_TOT], BF16, tag="cbf")
        cf32 = cpool.tile([128, C_TOT], F32, tag="cf32")
        ident = cpool.tile([128, 128], BF16, tag="ident")
        ones_r = cpool.tile([1, 128], F32, tag="ones_r")
        iota81 = cpool.tile([K, C], F32, tag="iota81")

        sres_l = [spool.tile([128, T, C], BF16, tag="sres", name=f"sres{i}")
                  for i in range(I)]
        lnA_l = [prepool.tile([128, T, K], BF16, tag="lnA", name=f"lnA{i}")
                 for i in range(I)]
        boxall = cpool.tile([1, I * 5 * K + 128], BF16, tag="boxall")
        qall = cpool.tile([128, I, 8 * NQ], BF16, tag="qall")
        lblall = cpool.tile([K, I], F32, tag="lblall")
        l4all = cpool.tile([128, I, T, 4], F32, tag="l4all")
        boxf_l = [boxall[:, i * 5 * K:(i + 1) * 5 * K]
                  .rearrange("p (a k) -> p a k", k=K) for i in range(I)]
        onesb = boxall[:, I * 5 * K:]
        qblk_l = [qall[:, i, :] for i in range(I)]
        lbl_l = [lblall[:, i:i + 1] for i in range(I)]

        # ACT queue: one merged box/onesb row (bb broadcasts unblock), scores 0
        nc.scalar.dma_start(out=boxall[:], in_=d_boxall[:, :])
        nc.scalar.dma_start(out=sres_l[0][:].rearrange("p t c -> p (t c)"),
                            in_=d_scores[0, :, :])
        # SP queue: jaccard consts, then everything else staged by need time
        nc.sync.dma_start(out=cbf[:], in_=d_cbf[:, :])
        nc.sync.dma_start(out=lnA_l[0][:].rearrange("p t k -> p (t k)"),
                          in_=d_lnA[0, :, :])
        nc.sync.dma_start(out=sres_l[1][:].rearrange("p t c -> p (t c)"),
                          in_=d_scores[1, :, :])
        nc.sync.dma_start(out=cf32[:], in_=d_cf32[:, :])
        nc.sync.dma_start(out=ident[:], in_=d_ident[:, :])
        nc.sync.dma_start(out=ones_r[:], in_=d_ones_r[:, :])
        nc.sync.dma_start(out=qall[:].rearrange("p i q -> p (i q)"),
                          in_=d_qall[:, :])
        nc.sync.dma_start(out=lblall[:], in_=d_lblall[:, :])
        nc.sync.dma_start(out=l4all[:].rearrange("p i t d -> p (i t d)"),
                          in_=d_locs[:, :])
        nc.sync.dma_start(out=iota81[:], in_=d_iota81[:, :])
        nc.sync.dma_start(out=lnA_l[1][:].rearrange("p t k -> p (t k)"),
                          in_=d_lnA[1, :, :])
        nc.sync.dma_start(out=sres_l[2][:].rearrange("p t c -> p (t c)"),
                          in_=d_scores[2, :, :])
        nc.sync.dma_start(out=lnA_l[2][:].rearrange("p t k -> p (t k)"),
                          in_=d_lnA[2, :, :])
        nc.sync.dma_start(out=sres_l[3][:].rearrange("p t c -> p (t c)"),
                          in_=d_scores[3, :, :])
        nc.sync.dma_start(out=lnA_l[3][:].rearrange("p t k -> p (t k)"),
                          in_=d_lnA[3, :, :])

        def prow(off):  # bf16 pre-expanded prior row view [128, T, K]
            return cbf[:, off:off + T * K].rearrange("p (t k) -> p t k", k=K)

        pxe = {nm: prow(off) for nm, off in
               [("px1", B_PX1), ("py1", B_PY1), ("px2", B_PX2),
                ("py2", B_PY2), ("parea", B_PAREA)]}
        kvb = prow(B_KV)
        eps_b = cbf[:, B_EPS:B_EPS + 1]
        io15 = cf32[:, C_IO15:C_IV4]
        iv4 = cf32[:, C_IV4:C_PC4].rearrange("p (t d) -> p t d", d=4)
        pc4 = cf32[:, C_PC4:C_ONE].rearrange("p (t d) -> p t d", d=4)
        ones_p = cf32[:, C_ONE:C_IDF]
        identf = cf32[:, C_IDF:C_O128]
        ones128 = cf32[:, C_O128:C_TOT]

        def rowsum(dst_row_ap, src_ap, n):
            """[P, n] f32 -> [1, n] partition sum written to dst_row_ap."""
            ps = pp_r.tile([128, 128], F32, tag="red_bc")
            nc.tensor.matmul(ps[0:1, :n], lhsT=ones_p[:src_ap.shape[0], :],
                             rhs=src_ap, start=True, stop=True)
            nc.vector.tensor_copy(dst_row_ap, ps[0:1, :n])

        def bcast_row(dst_ap, row_ap, n):
            """[1, n] f32 -> [128, n] replicated (dst may be bf16)."""
            ps = pp_r.tile([128, 128], F32, tag="red_bc")
            nc.tensor.matmul(ps[:, :n], lhsT=ones_r[:], rhs=row_ap,
                             start=True, stop=True)
            nc.vector.tensor_copy(dst_ap, ps[:, :n])

        def allreduce_sum(dst_ap, src_ap, n):
            ps = pp_r.tile([128, 128], F32, tag="red_bc")
            nc.tensor.matmul(ps[:, :n], lhsT=ones128, rhs=src_ap,
                             start=True, stop=True)
            nc.vector.tensor_copy(dst_ap, ps[:, :n])

        def maxreduce_row(dst_row_ap, src_ap, n):
            """[128, n] f32 -> [1, n] partition max written to dst_row_ap."""
            ps = pp_r.tile([128, 128], F32, tag="red_bc")
            nc.tensor.transpose(ps[:n, :], src_ap, identf)
            tsb = rpool.tile([128, 128], F32, tag="red_tsb")
            nc.vector.tensor_copy(tsb[:n, :], ps[:n, :])
            mx = rpool.tile([128, 1], F32, tag="red_mx")
            nc.vector.tensor_reduce(out=mx[:n, :], in_=tsb[:n, :],
                                    axis=AX.X, op=OP.max)
            ps2 = pp_r.tile([128, 128], F32, tag="red_bc")
            nc.tensor.transpose(ps2[0:1, :n], mx[:n, :], identf[:n, :n])
            nc.vector.tensor_copy(dst_row_ap, ps2[0:1, :n])

        # batched buffers [128, I, T]
        se4 = bpool.tile([128, I, T], BF16, tag="se4")
        lse4 = bpool.tile([128, I, T], F32, tag="lse4")
        cen4 = bpool.tile([128, I, T], BF16, tag="cen4")
        np4 = bpool.tile([128, I], F32, tag="np4")
        k34 = bpool.tile([128, I], F32, tag="k34")
        cnt = bpool.tile([128, I, 16], F32, tag="cnt")
        cntr = bpool.tile([128, I, 16], F32, tag="cntr")
        lo4 = bpool.tile([128, I], F32, tag="lo4")
        lop = bpool.tile([128, I], F32, tag="lop")
        hi4 = bpool.tile([128, I], F32, tag="hi4")
        scadd = bpool.tile([128, I, 4], F32, tag="scadd")   # fs, cn, lps, box
        scrow = bpool.tile([1, I, 4], F32, tag="scrow")
        bm4 = bpool.tile([128, I], F32, tag="bm4")
        bmrow = bpool.tile([1, I], F32, tag="bmrow")
        uf4 = bpool.tile([1, I], F32, tag="uf4")
        out_sb = bpool.tile([1, 16], F32, tag="out_sb")

        def emit_mining(i):
            # ---------------- mining (per image, pipelined) --------------
            nc.vector.tensor_scalar(out=k34[:, i:i + 1], in0=np4[:, i:i + 1],
                                    scalar1=3.0, scalar2=None, op0=OP.mult)
            allreduce_sum(cntr[:, i, :], cnt[:, i, :], 16)
            # lo = (#edges with count >= k) - 1   (edges j = 0..15)
            ge16 = bpool.tile([128, 16], F32, tag="ge16")
            nc.vector.tensor_scalar(out=ge16[:], in0=cntr[:, i, :],
                                    scalar1=k34[:, i:i + 1], scalar2=None,
                                    op0=OP.is_ge)
            nc.vector.tensor_reduce(out=lo4[:, i:i + 1], in_=ge16[:],
                                    axis=AX.X, op=OP.add)
            nc.vector.tensor_scalar(out=lo4[:, i:i + 1], in0=lo4[:, i:i + 1],
                                    scalar1=-1.0, scalar2=None, op0=OP.add)
            nc.vector.tensor_scalar(out=lop[:, i:i + 1], in0=lo4[:, i:i + 1],
                                    scalar1=1.0 / 16, scalar2=None, op0=OP.add)
            # level 2: thresholds lo + m/16 (io15 has (1..15)/16 then +999)
            thr2 = bpool.tile([128, 16], F32, tag="thr2")
            nc.vector.tensor_scalar(out=thr2[:], in0=io15,
                                    scalar1=lo4[:, i:i + 1], scalar2=None,
                                    op0=OP.add)
            c2 = bpool.tile([128, 16], F32, tag="c2")
            msci2 = wpool.tile([128, T], BF16, tag="msci2")
            for m in range(16):
                nc.vector.tensor_scalar(out=msci2[:], in0=cen4[:, i, :],
                                        scalar1=thr2[:, m:m + 1], scalar2=None,
                                        op0=OP.is_gt, op1=OP.add,
                                        accum_out=c2[:, m:m + 1])
            c2r = bpool.tile([128, 16], F32, tag="c2r")
            allreduce_sum(c2r[:], c2[:], 16)
            mc = bpool.tile([128, 1], F32, tag="mc")
            nc.vector.tensor_scalar(out=ge16[:], in0=c2r[:],
                                    scalar1=k34[:, i:i + 1], scalar2=None,
                                    op0=OP.is_ge, op1=OP.add, accum_out=mc[:])
            nc.vector.tensor_scalar(out=hi4[:, i:i + 1], in0=mc[:],
                                    scalar1=1.0 / 16, scalar2=lop[:, i:i + 1],
                                    op0=OP.mult, op1=OP.add)
            # F(hi), count(hi), boundary max
            fsc = bpool.tile([128, T], BF16, tag="fsc")
            nc.vector.scalar_tensor_tensor(
                out=fsc[:], in0=cen4[:, i, :], scalar=hi4[:, i:i + 1],
                in1=cen4[:, i, :], op0=OP.is_gt, op1=OP.mult,
                accum_out=scadd[:, i, 0:1])
            nc.vector.tensor_scalar(out=fsc[:], in0=cen4[:, i, :],
                                    scalar1=hi4[:, i:i + 1], scalar2=None,
                                    op0=OP.is_gt, op1=OP.add,
                                    accum_out=scadd[:, i, 1:2])
            nc.vector.scalar_tensor_tensor(
                out=fsc[:], in0=cen4[:, i, :], scalar=hi4[:, i:i + 1],
                in1=cen4[:, i, :], op0=OP.is_le, op1=OP.mult)
            nc.vector.tensor_reduce(out=bm4[:, i:i + 1], in_=fsc[:],
                                    axis=AX.X, op=OP.max)

        bb_l = []
        for i in range(I):
            bbt = prepool.tile([128, 5, K], BF16, tag="bb", name=f"bb{i}")
            ps_bb = pp_r.tile([128, 128], F32, tag="red_bc",
                              name=f"psbb{i}")
            nc.tensor.matmul(ps_bb[:, :5 * K], lhsT=onesb,
                             rhs=boxf_l[i].rearrange("p a k -> p (a k)"),
                             start=True, stop=True)
            nc.vector.tensor_copy(bbt[:].rearrange("p a k -> p (a k)"),
                                  ps_bb[:, :5 * K])
            bb_l.append(bbt)

        for i in range(I):
            sres = sres_l[i]
            l4 = l4all[:, i]
            qblk = qblk_l[i]
            lbl16 = lbl_l[i]
            bb = bb_l[i]

            def bcast_b(row):  # [128, K] box row -> [128, T, K] AP (packed k)
                return bb[:, row, :][:, None, :].broadcast_to([128, T, K])

            # ---------------- jaccard (log domain, bf16) -----------------
            # ordering uses i/(pa+ba): x/(A-x) is a monotone bijection of
            # x/A, so argmax/threshold semantics match IoU with thr ln(1/3)
            lov80 = wpool.tile([128, T8, K], BF16, tag="lov80")
            nc.vector.memset(lov80[:, T:, :], -100.0)
            lov = lov80[:, :T, :]
            ltx = wpool.tile([128, T, K], BF16, tag="ltx")
            lty = wpool.tile([128, T, K], BF16, tag="lty")
            w0 = wpool.tile([128, T, K], BF16, tag="w0")
            h0 = wpool.tile([128, T, K], BF16, tag="h0")
            wr = wpool.tile([128, T, K], BF16, tag="wr")
            hr = wpool.tile([128, T, K], BF16, tag="hr")
            inter = wpool.tile([128, T, K], BF16, tag="inter")

            nc.vector.tensor_tensor(out=ltx[:], in0=pxe["px1"],
                                    in1=bcast_b(0), op=OP.max)
            nc.vector.tensor_tensor(out=lty[:], in0=pxe["py1"],
                                    in1=bcast_b(1), op=OP.max)
            nc.vector.tensor_tensor(out=w0[:], in0=pxe["px2"],
                                    in1=bcast_b(2), op=OP.min)
            nc.vector.tensor_tensor(out=h0[:], in0=pxe["py2"],
                                    in1=bcast_b(3), op=OP.min)
            nc.vector.tensor_sub(wr[:], w0[:], ltx[:])
            nc.vector.tensor_sub(hr[:], h0[:], lty[:])
            nc.scalar.activation(wr[:], wr[:], ACTF.Relu)
            nc.scalar.activation(hr[:], hr[:], ACTF.Relu)
            nc.vector.tensor_mul(inter[:], wr[:], hr[:])
            nc.scalar.activation(inter[:], inter[:], ACTF.Ln, bias=eps_b)
            nc.vector.tensor_sub(lov, inter[:], lnA_l[i][:])

            # ---------------- matching pass 2 ----------------
            # per-box max over priors: dense max tree (80 = 2*2*2*2*5)
            tm1 = wpool.tile([128, 40, K], BF16, tag="tm1")
            nc.vector.tensor_tensor(out=tm1[:], in0=lov80[:, :40, :],
                                    in1=lov80[:, 40:, :], op=OP.max)
            tm2 = wpool.tile([128, 20, K], BF16, tag="tm2")
            nc.vector.tensor_tensor(out=tm2[:], in0=tm1[:, :20, :],
                                    in1=tm1[:, 20:, :], op=OP.max)
            tm3 = wpool.tile([128, 10, K], BF16, tag="tm3")
            nc.vector.tensor_tensor(out=tm3[:], in0=tm2[:, :10, :],
                                    in1=tm2[:, 10:, :], op=OP.max)
            tm4 = wpool.tile([128, 5, K], BF16, tag="tm4")
            nc.vector.tensor_tensor(out=tm4[:], in0=tm3[:, :5, :],
                                    in1=tm3[:, 5:, :], op=OP.max)
            m16 = wpool.tile([128, K], F32, tag="m16")
            nc.vector.tensor_reduce(
                out=m16[:], in_=tm4[:].rearrange("p t k -> p k t"),
                axis=AX.X, op=OP.max)
            m16row = wpool.tile([1, K], F32, tag="m16row")
            maxreduce_row(m16row[:], m16[:], K)
            m16rb = wpool.tile([128, K], BF16, tag="m16rb")
            bcast_row(m16rb[:], m16row[:], K)
            fmask = wpool.tile([128, T, K], BF16, tag="fmask")
            nc.vector.tensor_tensor(
                out=fmask[:], in0=lov,
                in1=m16rb[:][:, None, :].broadcast_to([128, T, K]),
                op=OP.is_equal)
            # uniform sentinel 104 (multi-forced priors go multi-hot; rare
            # and bounded): fm2 = fmask*204 - 100 in {-100, 104}, both 2x
            ovf = wpool.tile([128, T, K], BF16, tag="ovf")
            fm2 = wpool.tile([128, T, K], BF16, tag="fm2")
            nc.vector.tensor_scalar(out=fm2[:], in0=fmask[:],
                                    scalar1=204.0, scalar2=-100.0,
                                    op0=OP.mult, op1=OP.add)
            nc.vector.tensor_tensor(out=ovf[:], in0=fm2[:], in1=lov,
                                    op=OP.max)
            # per-prior max over k: dense tree on the packed innermost dim
            ms1 = wpool.tile([128, T, 8], BF16, tag="ms1")
            nc.vector.tensor_tensor(out=ms1[:], in0=ovf[:, :, 0:8],
                                    in1=ovf[:, :, 8:16], op=OP.max)
            ms2 = wpool.tile([128, T, 4], BF16, tag="ms2")
            nc.vector.tensor_tensor(out=ms2[:], in0=ms1[:, :, 0:4],
                                    in1=ms1[:, :, 4:8], op=OP.max)
            ms3 = wpool.tile([128, T, 2], BF16, tag="ms3")
            nc.vector.tensor_tensor(out=ms3[:], in0=ms2[:, :, 0:2],
                                    in1=ms2[:, :, 2:4], op=OP.max)
            pm = wpool.tile([128, T], BF16, tag="pm")
            nc.vector.tensor_tensor(out=pm[:], in0=ms3[:, :, 0],
                                    in1=ms3[:, :, 1], op=OP.max)
            # pmz = pm where positive else pm+1 (matches nothing): fuses the
            # one-hot and the pos mask into a single is_eq
            pmz = wpool.tile([128, T], BF16, tag="pmz")
            nc.vector.scalar_tensor_tensor(
                out=pmz[:], in0=pm[:], scalar=LN_THR, in1=pm[:],
                op0=OP.is_lt, op1=OP.add)
            wm72 = wpool.tile([128, T2 * K], BF16, tag="wm72")
            nc.vector.memset(wm72[:, T * K:], 0.0)
            wmat = wm72[:, :T * K].rearrange("p (t k) -> p t k", k=K)
            nc.vector.tensor_tensor(
                out=wmat, in0=ovf[:],
                in1=pmz[:][:, :, None].broadcast_to([128, T, K]),
                op=OP.is_equal)
            pos72 = wpool.tile([128, T2], F32, tag="pos72")
            nc.vector.memset(pos72[:, T:], 0.0)
            npt = wpool.tile([128, 1], F32, tag="npt")
            nc.vector.tensor_scalar(out=pos72[:, :T], in0=pm[:],
                                    scalar1=LN_THR, scalar2=None,
                                    op0=OP.is_ge, op1=OP.add, accum_out=npt[:])
            allreduce_sum(np4[:, i:i + 1], npt[:], 1)

            if i > 0:
                emit_mining(i - 1)

            # ---------------- box gather via PE ----------------
            ohT_ps = pp_t.tile([128, NB, 128], BF16, tag="ohT")
            for b in range(NB):
                nc.tensor.transpose(
                    ohT_ps[:, b, :],
                    wm72[:, b * 128:(b + 1) * 128],
                    ident[:])
            ohT_sb = wpool.tile([128, NB * 128], BF16, tag="ohT_sb")
            nc.scalar.copy(ohT_sb[:], ohT_ps[:].rearrange("p b n -> p (b n)"))

            sel_ps = pp_sel.tile([8 * NQ, NB, 128], F32, tag="sel")
            for b in range(NB):
                nc.tensor.matmul(sel_ps[:, b, :], lhsT=qblk,
                                 rhs=ohT_sb[:, b * 128:(b + 1) * 128],
                                 start=True, stop=True)
            sel_sb = wpool.tile([8 * NQ, NB * 128], BF16, tag="sel_sb")
            nc.scalar.copy(sel_sb[:], sel_ps[:].rearrange("p b n -> p (b n)"))
            bk_ps = pp_t.tile([128, NB, 8 * NQ], BF16, tag="ohT")
            for b in range(NB):
                nc.tensor.transpose(
                    bk_ps[:, b, :],
                    sel_sb[:, b * 128:(b + 1) * 128],
                    ident[:8 * NQ, :8 * NQ])
            selq = wpool.tile([128, NB * 8 * NQ], BF16, tag="selq")
            nc.scalar.copy(selq[:], bk_ps[:].rearrange("p b n -> p (b n)"))
            # selq[p, (blk*40 + tb*5 + q)] = sel_q at t = blk*8+tb
            sel4 = selq[:].rearrange("p (t q) -> p t q", q=NQ)[:, :, 0:4]

            # ---------------- box L1 (Pool chain + ACT abs-accum) ---------
            lp4 = wpool.tile([128, T, 4], F32, tag="lp4")
            nc.gpsimd.tensor_add(lp4[:], l4, pc4[:, :T, :])
            tb1 = wpool.tile([128, T2, 4], F32, tag="tb1")
            nc.vector.memset(tb1[:, T:, :], 0.0)
            nc.gpsimd.tensor_mul(tb1[:, :T, :], sel4[:, :T, :],
                                 iv4[:, :T, :])
            nc.gpsimd.tensor_sub(tb1[:, :T, :], lp4[:], tb1[:, :T, :])
            nc.vector.tensor_tensor(
                out=tb1[:, :T, :], in0=tb1[:, :T, :],
                in1=pos72[:, :T][:, :, None].broadcast_to([128, T, 4]),
                op=OP.mult)
            nc.scalar.activation(tb1[:], tb1[:], ACTF.Abs,
                                 accum_out=scadd[:, i, 3:4])

            # ---------------- U matrix (score at label) ----------------
            u_ps = pp_u.tile([K, C], F32, tag="u")
            for t_ in range(T):
                nc.tensor.matmul(u_ps[:], lhsT=wmat[:, t_, :],
                                 rhs=sres[:, t_, :],
                                 start=(t_ == 0), stop=(t_ == T - 1))
            u_sb = wpool.tile([K, C], F32, tag="u_sb")
            nc.scalar.copy(u_sb[:], u_ps[:])
            ufx = wpool.tile([K, C], F32, tag="ufx")
            ufa = wpool.tile([K, 1], F32, tag="ufa")
            nc.vector.scalar_tensor_tensor(
                out=ufx[:], in0=iota81[:], scalar=lbl16, in1=u_sb[:],
                op0=OP.is_equal, op1=OP.mult, accum_out=ufa[:])
            rowsum(uf4[:, i:i + 1], ufa[:], 1)

            # ------- CE: exp (ACT) + class sums (Pool half-adds + DVE) ----
            for ch in range(NCH):
                et = epool.tile([128, CT, C], BF16, tag="exps")
                nc.scalar.activation(
                    et[:], sres[:, ch * CT:(ch + 1) * CT, :], ACTF.Exp)
                et2 = epool.tile([128, CT, 40], BF16, tag="et2")
                et3 = epool.tile([128, CT, 20], BF16, tag="et3")
                sl = se4[:, i, ch * CT:(ch + 1) * CT]
                with nc.allow_low_precision("bf16 class sums"):
                    nc.vector.tensor_add(et2[:], et[:, :, 0:40],
                                         et[:, :, 40:80])
                    nc.vector.tensor_add(et3[:], et2[:, :, 0:20],
                                         et2[:, :, 20:40])
                    nc.vector.tensor_reduce(out=sl, in_=et3[:], axis=AX.X,
                                            op=OP.add)
                    nc.vector.tensor_add(sl, sl, et[:, :, 80])
            # lse per image, ce0, cen = (1-pos)*ce0 fused
            nc.scalar.activation(lse4[:, i, :], se4[:, i, :], ACTF.Ln)
            ce0 = wpool.tile([128, T], F32, tag="ce0")
            nc.vector.tensor_sub(ce0[:], lse4[:, i, :], sres[:, :, 0])
            nc.vector.scalar_tensor_tensor(
                out=cen4[:, i, :], in0=pos72[:, :T], scalar=0.5,
                in1=ce0[:], op0=OP.is_lt, op1=OP.mult)
            msci = wpool.tile([128, T], BF16, tag="msci")
            for j in range(16):
                nc.vector.tensor_scalar(out=msci[:], in0=cen4[:, i, :],
                                        scalar1=float(j), scalar2=None,
                                        op0=OP.is_gt, op1=OP.add,
                                        accum_out=cnt[:, i, j:j + 1])
            lpst = wpool.tile([128, T], F32, tag="lpst")
            nc.vector.scalar_tensor_tensor(
                out=lpst[:], in0=pos72[:, :T], scalar=1.0,
                in1=lse4[:, i, :], op0=OP.mult, op1=OP.mult,
                accum_out=scadd[:, i, 2:3])

        emit_mining(I - 1)

        rowsum(scrow[:].rearrange("p i s -> p (i s)"),
               scadd[:].rearrange("p i s -> p (i s)"), I * 4)
        maxreduce_row(bmrow[:], bm4[:], I)

        # ---------------- final combine (partition 0) ----------------
        r4 = bpool.tile([1, I], F32, tag="r4")
        nc.vector.tensor_sub(r4[:], k34[0:1, :], scrow[:, :, 1])
        nc.vector.tensor_mul(r4[:], r4[:], bmrow[:])
        nc.vector.tensor_add(r4[:], r4[:], scrow[:, :, 0])   # mine sums
        cep = bpool.tile([1, I], F32, tag="cep")
        nc.vector.tensor_sub(cep[:], scrow[:, :, 2], uf4[:])  # ce_pos sums
        nc.vector.tensor_copy(out_sb[:, 0:4], np4[0:1, :])
        nc.vector.tensor_copy(out_sb[:, 4:8], scrow[:, :, 3])
        nc.vector.tensor_copy(out_sb[:, 8:12], cep[:])
        nc.vector.tensor_copy(out_sb[:, 12:16], r4[:])
        nc.sync.dma_start(out=d_out[:, :], in_=out_sb[:])

    if fixup:
        _fixup_module(nc)
    return nc


def prepare_inputs(predicted_locs, predicted_scores, boxes, labels,
                   priors_centers):
    """Shard + marshal the full inputs into 8 per-core in_maps (p-major)."""
    predicted_locs = np.asarray(predicted_locs, np.float32)
    predicted_scores = np.asarray(predicted_scores, np.float32)
    boxes = np.asarray(boxes, np.float32)
    labels_f = np.asarray(labels).astype(np.float32)
    priors = np.asarray(priors_centers, np.float32)

    npad = PP - P
    # scores: pad rows have class0=0, others -50 -> lse=0, S0=0, ce0=0 exactly
    pad_scores = np.full((B, npad, C), -50.0, np.float32)
    pad_scores[:, :, 0] = 0.0
    scores_p = np.concatenate([predicted_scores, pad_scores], axis=1)
    # p-major: [B, 128, T*C]
    scores_pm = np.ascontiguousarray(
        scores_p.reshape(B, T, 128, C).transpose(0, 2, 1, 3)
    ).reshape(B, 128, T * C)
    scores_bf = _to_bf16(scores_pm)
    locs_p = np.concatenate(
        [predicted_locs, np.zeros((B, npad, 4), np.float32)], axis=1)
    locs_pm = np.ascontiguousarray(
        locs_p.reshape(B, T, 128, 4).transpose(0, 2, 1, 3)
    ).reshape(B, 128, T * 4)

    # prior rows pre-expanded across K (p-major, bf16)
    pad_pri = np.tile(np.array([-100.0, -100.0, 1.0, 1.0], np.float32),
                      (npad, 1))
    pri = np.concatenate([priors, pad_pri], axis=0)
    pcx, pcy, pw, ph = pri[:, 0], pri[:, 1], pri[:, 2], pri[:, 3]

    def pm_grid(v):  # [PP] -> [128, T]
        return np.ascontiguousarray(v.astype(np.float32).reshape(T, 128).T)

    def pexp(v):  # [PP] -> [128, T*K] expanded across K
        return np.repeat(pm_grid(v)[:, :, None], K, axis=2).reshape(128, T * K)

    kv = np.tile(KV0 + 0.5 * np.arange(K, dtype=np.float32), (128, T))
    eps_b = np.full((128, 1), 1e-20, np.float32)
    cbf = _to_bf16(np.concatenate(
        [pexp(pcx - pw / 2), pexp(pcy - ph / 2), pexp(pcx + pw / 2),
         pexp(pcy + ph / 2), pexp(pw * ph), kv, eps_b], axis=1))
    assert cbf.shape[1] == B_TOT

    # iv4/pc4 [128, T2, 4] host-assembled (d = x, y, w, h; tail zero)
    iv4 = np.zeros((128, T2, 4), np.float32)
    pc4 = np.zeros((128, T2, 4), np.float32)
    iv4[:, :T, 0] = pm_grid(10.0 / pw)
    iv4[:, :T, 1] = pm_grid(10.0 / ph)
    iv4[:, :T, 2] = 1.0
    iv4[:, :T, 3] = 1.0
    pc4[:, :T, 0] = pm_grid(pcx * (10.0 / pw))
    pc4[:, :T, 1] = pm_grid(pcy * (10.0 / ph))
    pc4[:, :T, 2] = pm_grid(5.0 * np.log(pw))
    pc4[:, :T, 3] = pm_grid(5.0 * np.log(ph))

    io15 = np.tile(np.concatenate([np.arange(1, 16, dtype=np.float32) / 16.0,
                                   [999.0]]), (128, 1))
    ones_p = np.ones((128, 1), np.float32)
    identf = np.eye(128, dtype=np.float32)
    cf32 = np.concatenate(
        [io15, iv4.reshape(128, T2 * 4), pc4.reshape(128, T2 * 4),
         ones_p, identf, np.ones((128, 128), np.float32)],
        axis=1).astype(np.float32)
    assert cf32.shape[1] == C_TOT

    bx1, by1, bx2, by2 = (boxes[:, :, d] for d in range(4))
    barea = (bx2 - bx1) * (by2 - by1)
    boxf = np.stack([bx1, by1, bx2, by2, barea], axis=1)      # [B, 5, K]
    # lnA[b, p, t, k] = ln(parea[p,t] + barea[b,k]) (p-major)
    parea_pm = pm_grid(pw * ph)                               # [128, T]
    lnA = _to_bf16(np.log(
        parea_pm[None, :, :, None] + barea[:, None, None, :].astype(np.float64)
    ).reshape(B, 128, T * K))
    q5 = np.stack([
        (bx1 + bx2) / 2, (by1 + by2) / 2,
        5.0 * np.log(bx2 - bx1), 5.0 * np.log(by2 - by1),
        np.zeros_like(bx1),
    ], axis=2).astype(np.float32)                           # [B, K, 5]
    qblk_f = np.zeros((B, 128, 8 * NQ), np.float32)
    for tb in range(8):
        qblk_f[:, tb * K:(tb + 1) * K, tb * NQ:(tb + 1) * NQ] = q5

    ident = _to_bf16(np.eye(128, dtype=np.float32))
    ones_r = np.ones((1, 128), np.float32)
    iota81 = np.tile(np.arange(C, dtype=np.float32), (K, 1))

    in_maps = []
    for c in range(NCORES):
        sl = slice(c * I, (c + 1) * I)
        boxall = _to_bf16(np.concatenate(
            [boxf[sl].reshape(-1), np.ones(128, np.float32)])[None, :])
        qall = _to_bf16(np.ascontiguousarray(
            qblk_f[sl].transpose(1, 0, 2)).reshape(128, I * 8 * NQ))
        lblall = np.ascontiguousarray(labels_f[sl].T)         # [K, I]
        locs2 = np.ascontiguousarray(
            locs_pm[sl].transpose(1, 0, 2)).reshape(128, I * T * 4)
        in_maps.append({
            "scores": scores_bf[sl],
            "locs": locs2,
            "cbf": cbf,
            "cf32": cf32,
            "boxall": boxall,
            "qall": qall,
            "lblall": lblall,
            "lnA": lnA[sl],
            "ident": ident,
            "ones_r": ones_r,
            "iota81": iota81,
        })
    return in_maps


def combine_outputs(outs):
    """outs: list of 8 per-core [1,16] arrays -> scalar loss."""
    parts = np.concatenate([o.reshape(4, 4) for o in outs], axis=1)  # [4, 32]
    n_pos_total = parts[0].sum()
    box_sum = parts[1].sum()
    class_sum = parts[2].sum() + parts[3].sum()
    loss = class_sum / n_pos_total + box_sum / (n_pos_total * 4.0)
    return np.float32(loss)


_NC_CACHE = {}


def kernel(predicted_locs, predicted_scores, boxes, labels, priors_centers):
    if "nc" not in _NC_CACHE:
        _NC_CACHE["nc"] = build_nc()
    nc = _NC_CACHE["nc"]
    in_maps = prepare_inputs(predicted_locs, predicted_scores, boxes, labels,
                             priors_centers)
    res = run_bass_kernel_spmd(nc, in_maps, list(range(NCORES)))
    outs = [res.results[c]["out"] for c in range(NCORES)]
    return combine_outputs(outs)


if __name__ == "__main__":
    import reference as R

    inputs = {k: np.asarray(v) for k, v in R.setup_inputs().items()}
    print("loss =", kernel(**inputs))
